# revision 23
# baseline (speedup 1.0000x reference)
"""GCN encoder (3x GCNConv: shared aggregation for mu/logstd) on 8 TRN2 NeuronCores.

Math: gcn_conv(x, A, W, b) = D^-1/2 (A+I) D^-1/2 (x W) + b, and the aggregation
commutes with the right matmul, so:
    y~ = dinv * (x @ W1)              (per-node row scale)
    h  = relu(dinv * SUM_edges y~[src] + b1)    (edge list includes self-loops)
    h~ = dinv * h
    g2 = dinv * SUM_edges h~[src]
    mu = g2 @ W_mu + b_mu ; logstd = g2 @ W_ls + b_ls

Sharding: nodes split contiguously across 8 cores (12500 each). Each core owns
the aggregation for its node range (dst-sharded). Gathered source rows come
from an AllGather'ed full y~ / h~ table (bf16), fetched with dma_gather using
int16 indices; the node space is split into 4 chunks of 25000 rows (both the
int16 range limit and the AllGather pipelining unit). Scatter-add is done by
one-hot matmuls accumulating in PSUM (S[e,d] = (dst_local[e]==d)).
"""
import numpy as np
import ml_dtypes
import concourse.bacc as bacc
import concourse.tile as tile
import concourse.bass as bass
import concourse.mybir as mybir
import concourse.bass_utils as bass_utils

N_CORES = 8
N_NODES = 100000
IN_C = 128
HID = 128
OUT_C = 64
R = N_NODES // N_CORES          # 12500 rows per core
NU = 4                          # src chunks / AllGather units
UR = R // NU                    # 3125 rows per unit per core
CHUNK = N_NODES // NU           # 25000 rows per (permuted) chunk
NBLK = (R + 127) // 128         # 98 dst blocks per core
BLK_GROUP = 16                  # dst blocks per PSUM group
GCOLS = 8                       # max 128-edge cols per dma_gather (1024 idxs = HW cap)

F32 = mybir.dt.float32
BF16 = mybir.dt.bfloat16
I16 = mybir.dt.int16
BF = ml_dtypes.bfloat16


def _wrap16(idx):
    """int16 indices -> [128, n/16] layout (16-partition wrap, replicated 8x)."""
    n = idx.shape[0]
    a = idx.astype(np.int16).reshape(n // 16, 16).T
    return np.ascontiguousarray(np.tile(a, (8, 1)))


def _prep(edge_index):
    """Host-side sharding prep: per-core padded edge streams + shared layout plan."""
    src = np.asarray(edge_index[0], dtype=np.int64)
    dst = np.asarray(edge_index[1], dtype=np.int64)
    # self-loops are NOT placed in the edge stream: their contribution
    # (dinv[d] * row[d]) is added in the epilogues from SBUF-local rows.
    # They still count toward the degree.
    deg = (np.bincount(dst, minlength=N_NODES) + 1).astype(np.float64)
    dinv = (1.0 / np.sqrt(deg)).astype(np.float32)

    # --- balanced node->position assignment -------------------------------
    # Reassign each node's position within its core (permutation within each
    # 3125-row AllGather unit, so source-chunk membership is unchanged) to
    # equalize the per-(block, chunk) edge counts: most segments then pack
    # into exactly ceil(mean/128) gather columns instead of paying the
    # max-over-cores Binomial tail.
    j_src = (src % R) // UR                     # source chunk (stable)
    cprof = np.bincount(dst * NU + j_src, minlength=N_NODES * NU)\
        .reshape(N_NODES, NU).astype(np.int64)  # per-dst-node chunk profile
    pos_of = np.empty(N_NODES, dtype=np.int64)  # node -> assigned local pos
    perm_of = []                                # per core: pos -> local node
    for k in range(N_CORES):
        pos_k = np.empty(R, dtype=np.int64)
        for u in range(NU):
            nodes = k * R + u * UR + np.arange(UR)
            P = cprof[nodes]                    # [UR, NU]
            lo, hi = u * UR, (u + 1) * UR
            b0, b1 = lo // 128, (hi + 127) // 128
            bins = [(max(128 * b, lo), min(128 * b + 128, hi))
                    for b in range(b0, b1)]
            cap = np.array([e - s for s, e in bins])
            # shared overflow blocks (cap 768 per cell) absorb each core's
            # Binomial excess so normal cells stay under 512 (w=4)
            ccap = np.array([768.0 if b % 24 == 23 else 512.0
                             for b in range(b0, b1)])
            fill = np.zeros(len(bins), dtype=np.int64)
            sums = np.zeros((len(bins), NU), dtype=np.int64)
            order = np.argsort(-P.sum(1), kind="stable")
            slot = np.empty(UR, dtype=np.int64)
            for i in order:
                ns = sums + P[i]
                hard = ns.max(axis=1) > ccap
                cost = (ns / ccap[:, None]).max(axis=1)
                cost[fill >= cap] = np.inf
                cost2 = np.where(hard, np.inf, cost)
                bsel = int(np.argmin(cost2))
                if not np.isfinite(cost2[bsel]):
                    bsel = int(np.argmin(cost))   # fallback: least overflow
                slot[i] = bins[bsel][0] + fill[bsel]
                fill[bsel] += 1
                sums[bsel] += P[i]
            pos_k[u * UR + np.arange(UR)] = slot
        pos_of[k * R:(k + 1) * R] = pos_k
        pk = np.empty(R, dtype=np.int64)
        pk[pos_k] = np.arange(R)
        perm_of.append(pk)                      # pos -> original local node

    # permuted (AllGather-major) source ids using ASSIGNED positions:
    # node at (core k, pos r=(j,i)) -> table row 25000j + 3125k + i
    k_of = src // R
    r_of = pos_of[src]
    j_of = r_of // UR
    i_of = r_of % UR
    psrc = CHUNK * j_of + UR * k_of + i_of
    c_of = psrc // CHUNK          # src chunk
    ci_of = psrc % CHUNK          # index within chunk (int16-safe, < 25000)

    kd = dst // R                 # owning core
    ld = pos_of[dst]
    b_of = ld // 128              # dst block
    dloc = ld % 128               # dst id within block

    g_of = b_of // BLK_GROUP      # block group
    # stream order: (core, group, chunk, block)
    order_key = ((kd * (NBLK // BLK_GROUP + 1) + g_of) * NU + c_of) * NBLK + b_of
    order = np.argsort(order_key, kind="stable")
    src_s, c_s, ci_s, kd_s, b_s, dloc_s = (
        a[order] for a in (src, c_of, ci_of, kd, b_of, dloc))

    # counts per (core, block, chunk) -> shared padded width w[b,c] (cols of 128)
    cnt = np.zeros((N_CORES, NBLK, NU), dtype=np.int64)
    np.add.at(cnt, (kd_s, b_s, c_s), 1)
    wmax = cnt.max(axis=0)                          # [NBLK, NU]
    w = ((wmax + 127) // 128).astype(np.int64)      # ceil; 0 stays 0

    # layout plan (shared across cores)
    ngroups = (NBLK + BLK_GROUP - 1) // BLK_GROUP
    col = 0
    seg = []          # (g, c, b, col_start, w_bc)
    for g in range(ngroups):
        blocks = range(g * BLK_GROUP, min((g + 1) * BLK_GROUP, NBLK))
        for c in range(NU):
            for b in blocks:
                if w[b, c] > 0:
                    seg.append((g, c, b, col, int(w[b, c])))
                    col += int(w[b, c])
    LT = col                                         # total 128-edge columns
    L = LT * 128

    # per-block first/last column (for PSUM start/stop flags)
    first_col = {}
    last_col = {}
    for (_g, _c, b, c0, wb) in seg:
        if b not in first_col:
            first_col[b] = c0
        last_col[b] = c0 + wb - 1
    block_of_col = np.full(LT, -1, dtype=np.int64)
    for (_g, _c, b, c0, wb) in seg:
        block_of_col[c0:c0 + wb] = b

    # gather calls: per (g, c) contiguous col range, split into <= GCOLS pieces
    calls = []        # (c, col_start, ncols)
    i = 0
    while i < len(seg):
        g, c = seg[i][0], seg[i][1]
        c0 = seg[i][3]
        cend = c0
        while i < len(seg) and seg[i][0] == g and seg[i][1] == c:
            cend = seg[i][3] + seg[i][4]
            i += 1
        p = c0
        while p < cend:
            n = min(GCOLS, cend - p)
            calls.append((c, p, n))
            p += n

    # per-core streams
    per_core = []
    # index into sorted stream: per (core, block, chunk) slice
    key_sorted = ((kd_s * (NBLK // BLK_GROUP + 1) + (b_s // BLK_GROUP)) * NU + c_s) * NBLK + b_s
    # boundaries via searchsorted on the sort keys
    for k in range(N_CORES):
        idx_arr = np.zeros(L, dtype=np.int16)
        dst_arr = np.full(L, -1.0, dtype=np.float32)
        sel = kd_s == k
        ci_k = ci_s[sel]
        b_k = b_s[sel]
        c_k = c_s[sel]
        dl_k = dloc_s[sel]
        g_k = b_k // BLK_GROUP
        key_k = (g_k * NU + c_k) * NBLK + b_k
        # stream is already sorted by key within core
        bounds = np.searchsorted(key_k, [(g * NU + c) * NBLK + b for (g, c, b, _c0, _w) in seg] +
                                 [(g * NU + c) * NBLK + b + 1 for (g, c, b, _c0, _w) in seg])
        nseg = len(seg)
        for si, (_g, _c, _b, c0, wb) in enumerate(seg):
            lo, hi = bounds[si], bounds[nseg + si]
            n = hi - lo
            assert n <= wb * 128
            # sort by source index within the segment: monotone HBM
            # addresses give the gather DMA row-buffer locality
            o = np.argsort(ci_k[lo:hi], kind="stable")
            idx_arr[c0 * 128: c0 * 128 + n] = ci_k[lo:hi][o]
            dst_arr[c0 * 128: c0 * 128 + n] = dl_k[lo:hi][o]
        # wrap idx into [128, L/16]; dst into [128, LT]
        idx16 = _wrap16(idx_arr)
        dst128 = np.ascontiguousarray(dst_arr.reshape(LT, 128).T.astype(BF))
        per_core.append((idx16, dst128, perm_of[k]))

    plan = dict(seg=seg, calls=calls, LT=LT, L=L,
                first_col=first_col, last_col=last_col, block_of_col=block_of_col)
    return dinv, plan, per_core


def _build(plan, reps=1, nocoll=False, cfg=None):
    """Build the SPMD Bass program (identical across cores).

    nocoll=True replaces collectives with local DMA copies (wrong values,
    same local-work shape) so TimelineSim / no-collective timing works.
    cfg: dict of tuning knobs (gbufs, sbufs, dense_gather, skip_s).
    """
    cfg = cfg or {}
    GBUFS = cfg.get("gbufs", 3)
    SBUFS = cfg.get("sbufs", 3)
    DENSE_GATHER = cfg.get("dense_gather", False)
    SKIP_S = cfg.get("skip_s", False)
    nc = bacc.Bacc("TRN2", target_bir_lowering=False, debug=False, num_devices=N_CORES,
                   num_swdge_queues=4)
    LT, L = plan["LT"], plan["L"]
    calls = plan["calls"]
    first_col, last_col = plan["first_col"], plan["last_col"]
    block_of_col = plan["block_of_col"]
    XCOLS = NBLK * 128  # zero-padded xT columns

    XF = NU * (CHUNK // 128 + 1) * 128  # full x, table order, per-chunk pad
    # inputs
    xT = nc.dram_tensor("xT", [128, XCOLS], BF16, kind="ExternalInput")
    xTf = nc.dram_tensor("xTf", [128, XF], BF16, kind="ExternalInput")
    dinvf_in = nc.dram_tensor("dinvf_in", [128, XF // 128], F32,
                              kind="ExternalInput")
    idx_in = nc.dram_tensor("idx_in", [128, L // 16], I16, kind="ExternalInput")
    dst_in = nc.dram_tensor("dst_in", [128, LT], BF16, kind="ExternalInput")
    dinv_in = nc.dram_tensor("dinv_in", [128, NBLK], F32, kind="ExternalInput")
    iota_in = nc.dram_tensor("iota_in", [128, 128], BF16, kind="ExternalInput")
    ident_in = nc.dram_tensor("ident_in", [128, 128], BF16, kind="ExternalInput")
    w1_in = nc.dram_tensor("w1_in", [128, HID], BF16, kind="ExternalInput")
    wmu_in = nc.dram_tensor("wmu_in", [HID, OUT_C], BF16, kind="ExternalInput")
    wls_in = nc.dram_tensor("wls_in", [HID, OUT_C], BF16, kind="ExternalInput")
    b1_in = nc.dram_tensor("b1_in", [128, HID], F32, kind="ExternalInput")
    bmuls_in = nc.dram_tensor("bmuls_in", [128, 2 * OUT_C], F32, kind="ExternalInput")
    # outputs
    mu_out = nc.dram_tensor("mu_out", [R, OUT_C], F32, kind="ExternalOutput")
    ls_out = nc.dram_tensor("ls_out", [R, OUT_C], F32, kind="ExternalOutput")

    def rows_of(b):
        return min(128, R - 128 * b)

    with tile.TileContext(nc) as tc:
        with (
            tc.tile_pool(name="const", bufs=1) as cpool,
            tc.tile_pool(name="xt", bufs=3) as xtp,
            tc.tile_pool(name="yh", bufs=4) as yhp,
            tc.tile_pool(name="gat", bufs=GBUFS) as gp,
            tc.tile_pool(name="sel", bufs=SBUFS) as sp,
            tc.tile_pool(name="epi", bufs=4) as ep,
            tc.tile_pool(name="psA", bufs=1, space="PSUM") as psA,
            tc.tile_pool(name="psB", bufs=2, space="PSUM") as psB,
            tc.tile_pool(name="dram", bufs=1, space="DRAM") as dram,
        ):
            # constants
            idx_sb = cpool.tile([128, L // 16], I16)
            dst_sb = cpool.tile([128, LT], BF16)
            dinv_sb = cpool.tile([128, NBLK], F32)
            iota_sb = cpool.tile([128, 128], BF16)
            ident_sb = cpool.tile([128, 128], BF16)
            w1_sb = cpool.tile([128, HID], BF16)
            wmu_sb = cpool.tile([HID, OUT_C], BF16)
            wls_sb = cpool.tile([HID, OUT_C], BF16)
            b1_sb = cpool.tile([128, HID], F32)
            bmuls_sb = cpool.tile([128, 2 * OUT_C], F32)
            # persistent local y~ / h~ rows (for the self-loop term)
            ylocal = cpool.tile([128, NBLK * HID], BF16)
            hlocal = cpool.tile([128, NBLK * HID], BF16)
            dinvf_sb = cpool.tile([128, XF // 128], F32)
            for sb, dr in ((idx_sb, idx_in), (dst_sb, dst_in), (dinv_sb, dinv_in),
                           (iota_sb, iota_in), (ident_sb, ident_in), (w1_sb, w1_in),
                           (wmu_sb, wmu_in), (wls_sb, wls_in), (b1_sb, b1_in),
                           (bmuls_sb, bmuls_in), (dinvf_sb, dinvf_in)):
                nc.sync.dma_start(out=sb[:], in_=dr.ap()[:])

            # internal DRAM
            h_in = dram.tile([R, HID], BF16)

            def alloc_full(pfx, shared=True):
                kw = dict(addr_space="Shared") if shared else {}
                return [dram.tile([CHUNK, HID], BF16, tag=f"{pfx}{j}",
                                  name=f"{pfx}{j}", **kw)
                        for j in range(NU)]

            # ---- phase 1a: local y~ rows (self-loop term only) ----
            def phase1_local():
                for b in range(NBLK):
                    xt_sb = xtp.tile([128, 128], BF16, tag="xt", name="xt_sb")
                    nc.sync.dma_start(out=xt_sb[:],
                                      in_=xT.ap()[:, 128 * b:128 * (b + 1)])
                    y_ps = psB.tile([128, HID], F32, tag="outps", space="PSUM",
                                    name="y_ps")
                    nc.tensor.matmul(out=y_ps[:], lhsT=xt_sb[:], rhs=w1_sb[:],
                                     start=True, stop=True)
                    nc.scalar.activation(out=ylocal[:, HID * b:HID * (b + 1)],
                                         in_=y_ps[:],
                                         func=mybir.ActivationFunctionType.Copy,
                                         scale=dinv_sb[:, b:b + 1])

            # ---- phase 1b: replicated full y~ table (replaces AllGather) ----
            CB = CHUNK // 128 + 1        # 196 column-blocks per chunk
            def phase1_full(y_full):
                for j in range(NU):
                    for bb in range(CB):
                        tb = CB * j + bb
                        xt_sb = xtp.tile([128, 128], BF16, tag="xt",
                                         name="xt_sb")
                        nc.sync.dma_start(
                            out=xt_sb[:],
                            in_=xTf.ap()[:, 128 * tb:128 * (tb + 1)])
                        y_ps = psB.tile([128, HID], F32, tag="outps",
                                        space="PSUM", name="y_ps")
                        nc.tensor.matmul(out=y_ps[:], lhsT=xt_sb[:],
                                         rhs=w1_sb[:], start=True, stop=True)
                        y_sb = yhp.tile([128, HID], BF16, tag="y", name="y_sb")
                        nc.scalar.activation(
                            out=y_sb[:], in_=y_ps[:],
                            func=mybir.ActivationFunctionType.Copy,
                            scale=dinvf_sb[:, tb:tb + 1])
                        rb = min(128, CHUNK - 128 * bb)
                        nc.sync.dma_start(
                            out=y_full[j][128 * bb:128 * bb + rb, :],
                            in_=y_sb[:rb, :])

            def agather(t_in, t_full):
                for j in range(NU):
                    if nocoll:
                        nc.sync.dma_start(out=t_full[j][0:UR, :],
                                          in_=t_in[UR * j:UR * (j + 1), :])
                    else:
                        nc.gpsimd.collective_compute(
                            "AllGather", mybir.AluOpType.bypass,
                            replica_groups=[list(range(N_CORES))],
                            ins=[t_in[UR * j:UR * (j + 1), :]],
                            outs=[t_full[j][:]],
                        )

            # per (group, bank): member blocks + first/last stream column.
            # PSUM start/stop must be bank-granular: start marks the whole 2KB
            # zero-region pending, so exactly one start and one stop per bank.
            ngroups = (NBLK + BLK_GROUP - 1) // BLK_GROUP
            bank_blocks = {}
            bank_first = {}
            bank_last = {}
            for b in range(NBLK):
                g, q = b // BLK_GROUP, (b % BLK_GROUP) // 4
                bank_blocks.setdefault((g, q), []).append(b)
                bank_first[(g, q)] = min(bank_first.get((g, q), 10 ** 9), first_col[b])
                bank_last[(g, q)] = max(bank_last.get((g, q), -1), last_col[b])
            def aggregation(table, epilogue):
                """Gather+one-hot-matmul aggregation over the shared edge stream."""
                acc = {}      # bank slot q -> psum tile
                done_blocks = set()
                for ci, (c, c0, ncols) in enumerate(calls):
                    g_tile = gp.tile([128, GCOLS, 128], BF16, tag="g")
                    if DENSE_GATHER:
                        nc.gpsimd.dma_start(
                            out=g_tile[:, 0:ncols, :],
                            in_=table[c][(c0 % 64) * 128:(c0 % 64) * 128 + 128 * ncols,
                                         :].rearrange("(w p) f -> p w f", p=128))
                    else:
                        nc.gpsimd.dma_gather(
                            out_ap=g_tile[:, 0:ncols, :],
                            in_ap=table[c][:],
                            idxs_ap=idx_sb[:, 8 * c0: 8 * (c0 + ncols)],
                            num_idxs=128 * ncols, num_idxs_reg=128 * ncols,
                            elem_size=128,
                            queue_num=ci % 4,
                            single_packet=cfg.get("single_packet", True),
                        )
                    s_tile = sp.tile([128, GCOLS, 128], BF16, tag="s")
                    if not SKIP_S:
                        dstap = dst_sb[:, c0:c0 + ncols].to_broadcast([128, ncols, 128])
                        iap = iota_sb[:]
                        iota_b = bass.AP(iap.tensor, iap.offset,
                                         [iap.ap[0], [0, ncols], iap.ap[1]])
                        nc.vector.tensor_tensor(out=s_tile[:, 0:ncols, :], in0=dstap,
                                                in1=iota_b, op=mybir.AluOpType.is_equal)
                    for t in range(ncols):
                        col = c0 + t
                        b = int(block_of_col[col])
                        g, q = b // BLK_GROUP, (b % BLK_GROUP) // 4
                        if col == bank_first[(g, q)]:
                            acc[q] = psA.tile([128, 512], F32, tag=f"acc{q}",
                                              name=f"acc{q}", space="PSUM")
                        a_ps = acc[q]
                        sl = slice(128 * (b % 4), 128 * (b % 4) + 128)
                        nc.tensor.matmul(out=a_ps[:, sl],
                                         lhsT=s_tile[:, t, :], rhs=g_tile[:, t, :],
                                         start=(col == bank_first[(g, q)]),
                                         stop=(col == bank_last[(g, q)]))
                        if col == bank_last[(g, q)]:
                            for bb in bank_blocks[(g, q)]:
                                sl2 = slice(128 * (bb % 4), 128 * (bb % 4) + 128)
                                epilogue(bb, a_ps[:, sl2])
                                done_blocks.add(bb)
                assert len(done_blocks) == NBLK

            # ---- agg1 epilogue: h~ = dinv * relu(dinv*(acc + y~) + b1) ----
            def epi1(b, acc_ap):
                # self-loop contribution: dinv_d^2 (x@W1)[d] = dinv_d * y~[d],
                # folded as (acc + y~[d]) * dinv_d
                t1 = ep.tile([128, HID], F32, tag="t1")
                nc.vector.tensor_tensor(out=t1[:], in0=acc_ap,
                                        in1=ylocal[:, HID * b:HID * (b + 1)],
                                        op=mybir.AluOpType.add)
                nc.vector.tensor_scalar_mul(out=t1[:], in0=t1[:],
                                            scalar1=dinv_sb[:, b:b + 1])
                nc.vector.tensor_tensor(out=t1[:], in0=t1[:], in1=b1_sb[:],
                                        op=mybir.AluOpType.add)
                h_sb = hlocal[:, HID * b:HID * (b + 1)]
                nc.scalar.activation(out=h_sb, in_=t1[:],
                                     func=mybir.ActivationFunctionType.Relu,
                                     scale=dinv_sb[:, b:b + 1])
                rb = rows_of(b)
                nc.sync.dma_start(out=h_in[128 * b:128 * b + rb, :], in_=h_sb[:rb, :])

            def first_half():
                phase1_local()
                y_full = alloc_full("y_full", shared=False)
                phase1_full(y_full)
                aggregation(y_full, epi1)
                h_full = alloc_full("h_full")
                agather(h_in, h_full)
                return h_full

            # ---- agg2 epilogue: g2 = dinv*(acc + dinv*h); mu/ls = g2 @ W + b ----
            def epi2(b, acc_ap):
                t2 = ep.tile([128, HID], F32, tag="t2")
                nc.vector.tensor_tensor(out=t2[:], in0=acc_ap,
                                        in1=hlocal[:, HID * b:HID * (b + 1)],
                                        op=mybir.AluOpType.add)
                g2_sb = ep.tile([128, HID], BF16, tag="g2")
                nc.scalar.activation(out=g2_sb[:], in_=t2[:],
                                     func=mybir.ActivationFunctionType.Copy,
                                     scale=dinv_sb[:, b:b + 1])
                tp_ps = psB.tile([128, HID], BF16, tag="tp", space="PSUM")
                nc.tensor.transpose(out=tp_ps[:], in_=g2_sb[:], identity=ident_sb[:])
                g2t_sb = ep.tile([128, HID], BF16, tag="g2t")
                nc.scalar.activation(out=g2t_sb[:], in_=tp_ps[:],
                                     func=mybir.ActivationFunctionType.Copy)
                o_ps = psB.tile([128, 2 * OUT_C], F32, tag="outps", space="PSUM")
                nc.tensor.matmul(out=o_ps[:, 0:OUT_C], lhsT=g2t_sb[:], rhs=wmu_sb[:],
                                 start=True, stop=True)
                nc.tensor.matmul(out=o_ps[:, OUT_C:2 * OUT_C], lhsT=g2t_sb[:],
                                 rhs=wls_sb[:], start=True, stop=True)
                muls_sb = ep.tile([128, 2 * OUT_C], F32, tag="muls")
                nc.vector.tensor_tensor(out=muls_sb[:], in0=o_ps[:], in1=bmuls_sb[:],
                                        op=mybir.AluOpType.add)
                rb = rows_of(b)
                nc.sync.dma_start(out=mu_out.ap()[128 * b:128 * b + rb, :],
                                  in_=muls_sb[:rb, 0:OUT_C])
                nc.sync.dma_start(out=ls_out.ap()[128 * b:128 * b + rb, :],
                                  in_=muls_sb[:rb, OUT_C:2 * OUT_C])

            for _ in range(reps):
                h_full = first_half()
                aggregation(h_full, epi2)

    nc.compile()
    return nc


TUNED_CFG = {"gbufs": 6, "sbufs": 6}


def build_in_maps(x, W1, b1, W_mu, b_mu, W_ls, b_ls, dinv, per_core):
    x = np.asarray(x)
    iota = np.tile(np.arange(128, dtype=np.float32), (128, 1)).astype(BF)
    ident = np.eye(128, dtype=np.float32).astype(BF)
    w1 = np.asarray(W1, np.float32).astype(BF)
    wmu = np.asarray(W_mu, np.float32).astype(BF)
    wls = np.asarray(W_ls, np.float32).astype(BF)
    b1t = np.tile(np.asarray(b1, np.float32), (128, 1))
    bmuls = np.tile(np.concatenate([np.asarray(b_mu, np.float32),
                                    np.asarray(b_ls, np.float32)]), (128, 1))
    XCOLS = NBLK * 128

    # replicated full x in table (chunk-major, assigned-position) order
    CB = CHUNK // 128 + 1
    XF = NU * CB * 128
    pa = np.stack([per_core[k][2] for k in range(N_CORES)])   # [cores, R]
    t = np.arange(N_NODES)
    jj, rem = t // CHUNK, t % CHUNK
    ks, ii = rem // UR, rem % UR
    node = ks * R + pa[ks, jj * UR + ii]
    xfull = x[node].astype(np.float32)
    dvfull = np.asarray(dinv, np.float32)[node]
    xTf = np.zeros((128, XF), dtype=BF)
    dinvf = np.ones((128, XF // 128), dtype=np.float32)
    for j in range(NU):
        seg = xfull[j * CHUNK:(j + 1) * CHUNK]
        xTf[:, CB * 128 * j:CB * 128 * j + CHUNK] = \
            np.ascontiguousarray(seg.T).astype(BF)
        pad = np.ones(CB * 128, np.float32)
        pad[:CHUNK] = dvfull[j * CHUNK:(j + 1) * CHUNK]
        dinvf[:, CB * j:CB * (j + 1)] = pad.reshape(CB, 128).T

    in_maps = []
    for k in range(N_CORES):
        idx16, dst128, perm = per_core[k]
        xk = x[R * k:R * (k + 1)][perm].astype(np.float32)
        xTk = np.zeros((128, XCOLS), dtype=BF)
        xTk[:, :R] = np.ascontiguousarray(xk.T).astype(BF)
        dv = dinv[R * k:R * (k + 1)][perm]
        padded = np.ones(NBLK * 128, dtype=np.float32)
        padded[:R] = dv
        dinv_blk = np.ascontiguousarray(padded.reshape(NBLK, 128).T)
        in_maps.append({
            "xT": xTk, "xTf": xTf, "dinvf_in": dinvf,
            "idx_in": idx16, "dst_in": dst128, "dinv_in": dinv_blk,
            "iota_in": iota, "ident_in": ident, "w1_in": w1, "wmu_in": wmu,
            "wls_in": wls, "b1_in": b1t, "bmuls_in": bmuls,
        })
    return in_maps


def kernel(x, edge_index, W1, b1, W_mu, b_mu, W_ls, b_ls):
    dinv, plan, per_core = _prep(np.asarray(edge_index))
    nc = _build(plan, cfg=TUNED_CFG)
    in_maps = build_in_maps(x, W1, b1, W_mu, b_mu, W_ls, b_ls, dinv, per_core)
    res = bass_utils.run_bass_kernel_spmd(nc, in_maps, core_ids=list(range(N_CORES)))
    mu = np.empty((N_NODES, OUT_C), dtype=np.float32)
    ls = np.empty((N_NODES, OUT_C), dtype=np.float32)
    for k in range(N_CORES):
        perm = per_core[k][2]
        mu[R * k + perm] = res.results[k]["mu_out"]
        ls[R * k + perm] = res.results[k]["ls_out"]
    return (mu, ls)



# revision 25
# speedup vs baseline: 1.3533x; 1.3533x over previous
"""GCN encoder (3x GCNConv: shared aggregation for mu/logstd) on 8 TRN2 NeuronCores.

Math: gcn_conv(x, A, W, b) = D^-1/2 (A+I) D^-1/2 (x W) + b, and the aggregation
commutes with the right matmul, so:
    y~ = dinv * (x @ W1)              (per-node row scale)
    h  = relu(dinv * SUM_edges y~[src] + b1)    (edge list includes self-loops)
    h~ = dinv * h
    g2 = dinv * SUM_edges h~[src]
    mu = g2 @ W_mu + b_mu ; logstd = g2 @ W_ls + b_ls

Sharding: nodes split contiguously across 8 cores (12500 each). Each core owns
the aggregation for its node range (dst-sharded). Gathered source rows come
from an AllGather'ed full y~ / h~ table (bf16), fetched with dma_gather using
int16 indices; the node space is split into 4 chunks of 25000 rows (both the
int16 range limit and the AllGather pipelining unit). Scatter-add is done by
one-hot matmuls accumulating in PSUM (S[e,d] = (dst_local[e]==d)).
"""
import numpy as np
import ml_dtypes
import concourse.bacc as bacc
import concourse.tile as tile
import concourse.bass as bass
import concourse.mybir as mybir
import concourse.bass_utils as bass_utils

N_CORES = 8
N_NODES = 100000
IN_C = 128
HID = 128
OUT_C = 64
R = N_NODES // N_CORES          # 12500 rows per core
NU = 4                          # src chunks / AllGather units
UR = R // NU                    # 3125 rows per unit per core
CHUNK = N_NODES // NU           # 25000 rows per (permuted) chunk
NBLK = (R + 127) // 128         # 98 dst blocks per core
BLK_GROUP = 16                  # dst blocks per PSUM group
GCOLS = 8                       # max 128-edge cols per dma_gather (1024 idxs = HW cap)

F32 = mybir.dt.float32
BF16 = mybir.dt.bfloat16
I16 = mybir.dt.int16
BF = ml_dtypes.bfloat16


def _wrap16(idx):
    """int16 indices -> [128, n/16] layout (16-partition wrap, replicated 8x)."""
    n = idx.shape[0]
    a = idx.astype(np.int16).reshape(n // 16, 16).T
    return np.ascontiguousarray(np.tile(a, (8, 1)))


def _prep(edge_index):
    """Host-side sharding prep: per-core padded edge streams + shared layout plan."""
    src = np.asarray(edge_index[0], dtype=np.int64)
    dst = np.asarray(edge_index[1], dtype=np.int64)
    # self-loops are NOT placed in the edge stream: their contribution
    # (dinv[d] * row[d]) is added in the epilogues from SBUF-local rows.
    # They still count toward the degree.
    deg = (np.bincount(dst, minlength=N_NODES) + 1).astype(np.float64)
    dinv = (1.0 / np.sqrt(deg)).astype(np.float32)

    # --- balanced node->position assignment -------------------------------
    # Reassign each node's position within its core (permutation within each
    # 3125-row AllGather unit, so source-chunk membership is unchanged) to
    # equalize the per-(block, chunk) edge counts: most segments then pack
    # into exactly ceil(mean/128) gather columns instead of paying the
    # max-over-cores Binomial tail.
    j_src = (src % R) // UR                     # source chunk (stable)
    cprof = np.bincount(dst * NU + j_src, minlength=N_NODES * NU)\
        .reshape(N_NODES, NU).astype(np.int64)  # per-dst-node chunk profile
    pos_of = np.empty(N_NODES, dtype=np.int64)  # node -> assigned local pos
    perm_of = []                                # per core: pos -> local node
    for k in range(N_CORES):
        pos_k = np.empty(R, dtype=np.int64)
        for u in range(NU):
            nodes = k * R + u * UR + np.arange(UR)
            P = cprof[nodes]                    # [UR, NU]
            lo, hi = u * UR, (u + 1) * UR
            b0, b1 = lo // 128, (hi + 127) // 128
            bins = [(max(128 * b, lo), min(128 * b + 128, hi))
                    for b in range(b0, b1)]
            cap = np.array([e - s for s, e in bins])
            # shared overflow blocks (cap 768 per cell) absorb each core's
            # Binomial excess so normal cells stay under 512 (w=4)
            ccap = np.array([768.0 if b % 24 == 23 else 512.0
                             for b in range(b0, b1)])
            fill = np.zeros(len(bins), dtype=np.int64)
            sums = np.zeros((len(bins), NU), dtype=np.int64)
            order = np.argsort(-P.sum(1), kind="stable")
            slot = np.empty(UR, dtype=np.int64)
            for i in order:
                ns = sums + P[i]
                hard = ns.max(axis=1) > ccap
                cost = (ns / ccap[:, None]).max(axis=1)
                cost[fill >= cap] = np.inf
                cost2 = np.where(hard, np.inf, cost)
                bsel = int(np.argmin(cost2))
                if not np.isfinite(cost2[bsel]):
                    bsel = int(np.argmin(cost))   # fallback: least overflow
                slot[i] = bins[bsel][0] + fill[bsel]
                fill[bsel] += 1
                sums[bsel] += P[i]
            pos_k[u * UR + np.arange(UR)] = slot
        pos_of[k * R:(k + 1) * R] = pos_k
        pk = np.empty(R, dtype=np.int64)
        pk[pos_k] = np.arange(R)
        perm_of.append(pk)                      # pos -> original local node

    # permuted (AllGather-major) source ids using ASSIGNED positions:
    # node at (core k, pos r=(j,i)) -> table row 25000j + 3125k + i
    k_of = src // R
    r_of = pos_of[src]
    j_of = r_of // UR
    i_of = r_of % UR
    psrc = CHUNK * j_of + UR * k_of + i_of
    c_of = psrc // CHUNK          # src chunk
    ci_of = psrc % CHUNK          # index within chunk (int16-safe, < 25000)

    kd = dst // R                 # owning core
    ld = pos_of[dst]
    b_of = ld // 128              # dst block
    dloc = ld % 128               # dst id within block

    g_of = b_of // BLK_GROUP      # block group
    # stream order: (core, group, chunk, block)
    order_key = ((kd * (NBLK // BLK_GROUP + 1) + g_of) * NU + c_of) * NBLK + b_of
    order = np.argsort(order_key, kind="stable")
    src_s, c_s, ci_s, kd_s, b_s, dloc_s = (
        a[order] for a in (src, c_of, ci_of, kd, b_of, dloc))

    # counts per (core, block, chunk) -> shared padded width w[b,c] (cols of 128)
    cnt = np.zeros((N_CORES, NBLK, NU), dtype=np.int64)
    np.add.at(cnt, (kd_s, b_s, c_s), 1)
    wmax = cnt.max(axis=0)                          # [NBLK, NU]
    w = ((wmax + 127) // 128).astype(np.int64)      # ceil; 0 stays 0

    # layout plan (shared across cores)
    ngroups = (NBLK + BLK_GROUP - 1) // BLK_GROUP
    col = 0
    seg = []          # (g, c, b, col_start, w_bc)
    for g in range(ngroups):
        blocks = range(g * BLK_GROUP, min((g + 1) * BLK_GROUP, NBLK))
        for c in range(NU):
            for b in blocks:
                if w[b, c] > 0:
                    seg.append((g, c, b, col, int(w[b, c])))
                    col += int(w[b, c])
    LT = col                                         # total 128-edge columns
    L = LT * 128

    # per-block first/last column (for PSUM start/stop flags)
    first_col = {}
    last_col = {}
    for (_g, _c, b, c0, wb) in seg:
        if b not in first_col:
            first_col[b] = c0
        last_col[b] = c0 + wb - 1
    block_of_col = np.full(LT, -1, dtype=np.int64)
    for (_g, _c, b, c0, wb) in seg:
        block_of_col[c0:c0 + wb] = b

    # gather calls: per (g, c) contiguous col range, split into <= GCOLS pieces
    calls = []        # (c, col_start, ncols)
    i = 0
    while i < len(seg):
        g, c = seg[i][0], seg[i][1]
        c0 = seg[i][3]
        cend = c0
        while i < len(seg) and seg[i][0] == g and seg[i][1] == c:
            cend = seg[i][3] + seg[i][4]
            i += 1
        p = c0
        while p < cend:
            n = min(GCOLS, cend - p)
            calls.append((c, p, n))
            p += n

    # per-core streams
    per_core = []
    # index into sorted stream: per (core, block, chunk) slice
    key_sorted = ((kd_s * (NBLK // BLK_GROUP + 1) + (b_s // BLK_GROUP)) * NU + c_s) * NBLK + b_s
    # boundaries via searchsorted on the sort keys
    for k in range(N_CORES):
        idx_arr = np.zeros(L, dtype=np.int16)
        dst_arr = np.full(L, -1.0, dtype=np.float32)
        sel = kd_s == k
        ci_k = ci_s[sel]
        b_k = b_s[sel]
        c_k = c_s[sel]
        dl_k = dloc_s[sel]
        g_k = b_k // BLK_GROUP
        key_k = (g_k * NU + c_k) * NBLK + b_k
        # stream is already sorted by key within core
        bounds = np.searchsorted(key_k, [(g * NU + c) * NBLK + b for (g, c, b, _c0, _w) in seg] +
                                 [(g * NU + c) * NBLK + b + 1 for (g, c, b, _c0, _w) in seg])
        nseg = len(seg)
        for si, (_g, _c, _b, c0, wb) in enumerate(seg):
            lo, hi = bounds[si], bounds[nseg + si]
            n = hi - lo
            assert n <= wb * 128
            # sort by source index within the segment: monotone HBM
            # addresses give the gather DMA row-buffer locality
            o = np.argsort(ci_k[lo:hi], kind="stable")
            idx_arr[c0 * 128: c0 * 128 + n] = ci_k[lo:hi][o]
            dst_arr[c0 * 128: c0 * 128 + n] = dl_k[lo:hi][o]
        # wrap idx into [128, L/16]; dst into [128, LT]
        idx16 = _wrap16(idx_arr)
        dst128 = np.ascontiguousarray(dst_arr.reshape(LT, 128).T.astype(BF))
        per_core.append((idx16, dst128, perm_of[k]))

    plan = dict(seg=seg, calls=calls, LT=LT, L=L,
                first_col=first_col, last_col=last_col, block_of_col=block_of_col)
    return dinv, plan, per_core


def _build(plan, reps=1, nocoll=False, cfg=None):
    """Build the SPMD Bass program (identical across cores).

    nocoll=True replaces collectives with local DMA copies (wrong values,
    same local-work shape) so TimelineSim / no-collective timing works.
    cfg: dict of tuning knobs (gbufs, sbufs, dense_gather, skip_s).
    """
    cfg = cfg or {}
    GBUFS = cfg.get("gbufs", 3)
    SBUFS = cfg.get("sbufs", 3)
    DENSE_GATHER = cfg.get("dense_gather", False)
    SKIP_S = cfg.get("skip_s", False)
    nc = bacc.Bacc("TRN2", target_bir_lowering=False, debug=False, num_devices=N_CORES,
                   num_swdge_queues=4)
    LT, L = plan["LT"], plan["L"]
    calls = plan["calls"]
    first_col, last_col = plan["first_col"], plan["last_col"]
    block_of_col = plan["block_of_col"]
    XCOLS = NBLK * 128  # zero-padded xT columns

    XF = NU * (CHUNK // 128 + 1) * 128  # full x, table order, per-chunk pad
    # inputs
    xT = nc.dram_tensor("xT", [128, XCOLS], BF16, kind="ExternalInput")
    xTf = nc.dram_tensor("xTf", [128, XF], BF16, kind="ExternalInput")
    dinvf_in = nc.dram_tensor("dinvf_in", [128, XF // 128], F32,
                              kind="ExternalInput")
    idx_in = nc.dram_tensor("idx_in", [128, L // 16], I16, kind="ExternalInput")
    dst_in = nc.dram_tensor("dst_in", [128, LT], BF16, kind="ExternalInput")
    dinv_in = nc.dram_tensor("dinv_in", [128, NBLK], F32, kind="ExternalInput")
    iota_in = nc.dram_tensor("iota_in", [128, 128], BF16, kind="ExternalInput")
    ident_in = nc.dram_tensor("ident_in", [128, 128], BF16, kind="ExternalInput")
    w1_in = nc.dram_tensor("w1_in", [128, HID], BF16, kind="ExternalInput")
    wmu_in = nc.dram_tensor("wmu_in", [HID, OUT_C], BF16, kind="ExternalInput")
    wls_in = nc.dram_tensor("wls_in", [HID, OUT_C], BF16, kind="ExternalInput")
    b1_in = nc.dram_tensor("b1_in", [128, HID], F32, kind="ExternalInput")
    bmuls_in = nc.dram_tensor("bmuls_in", [128, 2 * OUT_C], F32, kind="ExternalInput")
    # outputs
    mu_out = nc.dram_tensor("mu_out", [R, OUT_C], F32, kind="ExternalOutput")
    ls_out = nc.dram_tensor("ls_out", [R, OUT_C], F32, kind="ExternalOutput")

    def rows_of(b):
        return min(128, R - 128 * b)

    with tile.TileContext(nc) as tc:
        with (
            tc.tile_pool(name="const", bufs=1) as cpool,
            tc.tile_pool(name="xt", bufs=3) as xtp,
            tc.tile_pool(name="yh", bufs=4) as yhp,
            tc.tile_pool(name="gat", bufs=GBUFS) as gp,
            tc.tile_pool(name="sel", bufs=SBUFS) as sp,
            tc.tile_pool(name="epi", bufs=4) as ep,
            tc.tile_pool(name="psA", bufs=1, space="PSUM") as psA,
            tc.tile_pool(name="psB", bufs=2, space="PSUM") as psB,
            tc.tile_pool(name="dram", bufs=1, space="DRAM") as dram,
        ):
            # constants
            idx_sb = cpool.tile([128, L // 16], I16)
            dst_sb = cpool.tile([128, LT], BF16)
            dinv_sb = cpool.tile([128, NBLK], F32)
            iota_sb = cpool.tile([128, 128], BF16)
            ident_sb = cpool.tile([128, 128], BF16)
            w1_sb = cpool.tile([128, HID], BF16)
            wmu_sb = cpool.tile([HID, OUT_C], BF16)
            wls_sb = cpool.tile([HID, OUT_C], BF16)
            b1_sb = cpool.tile([128, HID], F32)
            bmuls_sb = cpool.tile([128, 2 * OUT_C], F32)
            # persistent local y~ / h~ rows (for the self-loop term)
            ylocal = cpool.tile([128, NBLK * HID], BF16)
            hlocal = cpool.tile([128, NBLK * HID], BF16)
            dinvf_sb = cpool.tile([128, XF // 128], F32)
            for sb, dr in ((idx_sb, idx_in), (dst_sb, dst_in), (dinv_sb, dinv_in),
                           (iota_sb, iota_in), (ident_sb, ident_in), (w1_sb, w1_in),
                           (wmu_sb, wmu_in), (wls_sb, wls_in), (b1_sb, b1_in),
                           (bmuls_sb, bmuls_in), (dinvf_sb, dinvf_in)):
                nc.sync.dma_start(out=sb[:], in_=dr.ap()[:])

            # internal DRAM
            h_in = dram.tile([R, HID], BF16)

            def alloc_full(pfx, shared=True):
                kw = dict(addr_space="Shared") if shared else {}
                return [dram.tile([CHUNK, HID], BF16, tag=f"{pfx}{j}",
                                  name=f"{pfx}{j}", **kw)
                        for j in range(NU)]

            # ---- phase 1a: local y~ rows (self-loop term only) ----
            def phase1_local():
                for b in range(NBLK):
                    xt_sb = xtp.tile([128, 128], BF16, tag="xt", name="xt_sb")
                    nc.sync.dma_start(out=xt_sb[:],
                                      in_=xT.ap()[:, 128 * b:128 * (b + 1)])
                    y_ps = psB.tile([128, HID], F32, tag="outps", space="PSUM",
                                    name="y_ps")
                    nc.tensor.matmul(out=y_ps[:], lhsT=xt_sb[:], rhs=w1_sb[:],
                                     start=True, stop=True)
                    nc.scalar.activation(out=ylocal[:, HID * b:HID * (b + 1)],
                                         in_=y_ps[:],
                                         func=mybir.ActivationFunctionType.Copy,
                                         scale=dinv_sb[:, b:b + 1])

            # ---- phase 1b: replicated full y~ table (replaces AllGather) ----
            # slab-batched DMA: 512KB loads/writes, not per-128-row transfers
            CB = CHUNK // 128 + 1        # 196 column-blocks per chunk
            SLAB = 16
            def phase1_full(y_full):
                for j in range(NU):
                    for s0 in range(0, CB, SLAB):
                        sn = min(SLAB, CB - s0)
                        xs = xtp.tile([128, SLAB * 128], BF16, tag="xs",
                                      name="xs")
                        nc.sync.dma_start(
                            out=xs[:, :sn * 128],
                            in_=xTf.ap()[:, 128 * (CB * j + s0):
                                         128 * (CB * j + s0 + sn)])
                        ys = yhp.tile([128, SLAB * HID], BF16, tag="ys",
                                      name="ys")
                        for q in range(sn):
                            tb = CB * j + s0 + q
                            y_ps = psB.tile([128, HID], F32, tag="outps",
                                            space="PSUM", name="y_ps")
                            nc.tensor.matmul(out=y_ps[:],
                                             lhsT=xs[:, 128 * q:128 * (q + 1)],
                                             rhs=w1_sb[:], start=True, stop=True)
                            nc.scalar.activation(
                                out=ys[:, HID * q:HID * (q + 1)], in_=y_ps[:],
                                func=mybir.ActivationFunctionType.Copy,
                                scale=dinvf_sb[:, tb:tb + 1])
                        rows = min(128 * sn, CHUNK - 128 * s0)
                        fw = rows // 128
                        if fw:
                            ysap = ys[:]
                            nc.sync.dma_start(
                                out=y_full[j][128 * s0:128 * s0 + 128 * fw, :]
                                    .rearrange("(w p) f -> p w f", p=128),
                                in_=bass.AP(ysap.tensor, ysap.offset,
                                            [ysap.ap[0], [HID, fw], [1, HID]]))
                        tail = rows - 128 * fw
                        if tail:
                            nc.sync.dma_start(
                                out=y_full[j][128 * s0 + 128 * fw:
                                              128 * s0 + rows, :],
                                in_=ys[:tail, HID * fw:HID * (fw + 1)])

            def agather(t_in, t_full):
                for j in range(NU):
                    if nocoll:
                        nc.sync.dma_start(out=t_full[j][0:UR, :],
                                          in_=t_in[UR * j:UR * (j + 1), :])
                    else:
                        nc.gpsimd.collective_compute(
                            "AllGather", mybir.AluOpType.bypass,
                            replica_groups=[list(range(N_CORES))],
                            ins=[t_in[UR * j:UR * (j + 1), :]],
                            outs=[t_full[j][:]],
                        )

            # per (group, bank): member blocks + first/last stream column.
            # PSUM start/stop must be bank-granular: start marks the whole 2KB
            # zero-region pending, so exactly one start and one stop per bank.
            ngroups = (NBLK + BLK_GROUP - 1) // BLK_GROUP
            bank_blocks = {}
            bank_first = {}
            bank_last = {}
            for b in range(NBLK):
                g, q = b // BLK_GROUP, (b % BLK_GROUP) // 4
                bank_blocks.setdefault((g, q), []).append(b)
                bank_first[(g, q)] = min(bank_first.get((g, q), 10 ** 9), first_col[b])
                bank_last[(g, q)] = max(bank_last.get((g, q), -1), last_col[b])
            def aggregation(table, epilogue):
                """Gather+one-hot-matmul aggregation over the shared edge stream."""
                acc = {}      # bank slot q -> psum tile
                done_blocks = set()
                for ci, (c, c0, ncols) in enumerate(calls):
                    g_tile = gp.tile([128, GCOLS, 128], BF16, tag="g")
                    if DENSE_GATHER:
                        nc.gpsimd.dma_start(
                            out=g_tile[:, 0:ncols, :],
                            in_=table[c][(c0 % 64) * 128:(c0 % 64) * 128 + 128 * ncols,
                                         :].rearrange("(w p) f -> p w f", p=128))
                    else:
                        nc.gpsimd.dma_gather(
                            out_ap=g_tile[:, 0:ncols, :],
                            in_ap=table[c][:],
                            idxs_ap=idx_sb[:, 8 * c0: 8 * (c0 + ncols)],
                            num_idxs=128 * ncols, num_idxs_reg=128 * ncols,
                            elem_size=128,
                            queue_num=ci % 4,
                            single_packet=cfg.get("single_packet", True),
                        )
                    s_tile = sp.tile([128, GCOLS, 128], BF16, tag="s")
                    if not SKIP_S:
                        dstap = dst_sb[:, c0:c0 + ncols].to_broadcast([128, ncols, 128])
                        iap = iota_sb[:]
                        iota_b = bass.AP(iap.tensor, iap.offset,
                                         [iap.ap[0], [0, ncols], iap.ap[1]])
                        nc.vector.tensor_tensor(out=s_tile[:, 0:ncols, :], in0=dstap,
                                                in1=iota_b, op=mybir.AluOpType.is_equal)
                    for t in range(ncols):
                        col = c0 + t
                        b = int(block_of_col[col])
                        g, q = b // BLK_GROUP, (b % BLK_GROUP) // 4
                        if col == bank_first[(g, q)]:
                            acc[q] = psA.tile([128, 512], F32, tag=f"acc{q}",
                                              name=f"acc{q}", space="PSUM")
                        a_ps = acc[q]
                        sl = slice(128 * (b % 4), 128 * (b % 4) + 128)
                        nc.tensor.matmul(out=a_ps[:, sl],
                                         lhsT=s_tile[:, t, :], rhs=g_tile[:, t, :],
                                         start=(col == bank_first[(g, q)]),
                                         stop=(col == bank_last[(g, q)]))
                        if col == bank_last[(g, q)]:
                            for bb in bank_blocks[(g, q)]:
                                sl2 = slice(128 * (bb % 4), 128 * (bb % 4) + 128)
                                epilogue(bb, a_ps[:, sl2])
                                done_blocks.add(bb)
                assert len(done_blocks) == NBLK

            # ---- agg1 epilogue: h~ = dinv * relu(dinv*(acc + y~) + b1) ----
            def epi1(b, acc_ap):
                # self-loop contribution: dinv_d^2 (x@W1)[d] = dinv_d * y~[d],
                # folded as (acc + y~[d]) * dinv_d
                t1 = ep.tile([128, HID], F32, tag="t1")
                nc.vector.tensor_tensor(out=t1[:], in0=acc_ap,
                                        in1=ylocal[:, HID * b:HID * (b + 1)],
                                        op=mybir.AluOpType.add)
                nc.vector.tensor_scalar_mul(out=t1[:], in0=t1[:],
                                            scalar1=dinv_sb[:, b:b + 1])
                nc.vector.tensor_tensor(out=t1[:], in0=t1[:], in1=b1_sb[:],
                                        op=mybir.AluOpType.add)
                h_sb = hlocal[:, HID * b:HID * (b + 1)]
                nc.scalar.activation(out=h_sb, in_=t1[:],
                                     func=mybir.ActivationFunctionType.Relu,
                                     scale=dinv_sb[:, b:b + 1])
                rb = rows_of(b)
                nc.sync.dma_start(out=h_in[128 * b:128 * b + rb, :], in_=h_sb[:rb, :])

            def first_half():
                phase1_local()
                y_full = alloc_full("y_full", shared=False)
                phase1_full(y_full)
                aggregation(y_full, epi1)
                h_full = alloc_full("h_full")
                agather(h_in, h_full)
                return h_full

            # ---- agg2 epilogue: g2 = dinv*(acc + dinv*h); mu/ls = g2 @ W + b ----
            def epi2(b, acc_ap):
                t2 = ep.tile([128, HID], F32, tag="t2")
                nc.vector.tensor_tensor(out=t2[:], in0=acc_ap,
                                        in1=hlocal[:, HID * b:HID * (b + 1)],
                                        op=mybir.AluOpType.add)
                g2_sb = ep.tile([128, HID], BF16, tag="g2")
                nc.scalar.activation(out=g2_sb[:], in_=t2[:],
                                     func=mybir.ActivationFunctionType.Copy,
                                     scale=dinv_sb[:, b:b + 1])
                tp_ps = psB.tile([128, HID], BF16, tag="tp", space="PSUM")
                nc.tensor.transpose(out=tp_ps[:], in_=g2_sb[:], identity=ident_sb[:])
                g2t_sb = ep.tile([128, HID], BF16, tag="g2t")
                nc.scalar.activation(out=g2t_sb[:], in_=tp_ps[:],
                                     func=mybir.ActivationFunctionType.Copy)
                o_ps = psB.tile([128, 2 * OUT_C], F32, tag="outps", space="PSUM")
                nc.tensor.matmul(out=o_ps[:, 0:OUT_C], lhsT=g2t_sb[:], rhs=wmu_sb[:],
                                 start=True, stop=True)
                nc.tensor.matmul(out=o_ps[:, OUT_C:2 * OUT_C], lhsT=g2t_sb[:],
                                 rhs=wls_sb[:], start=True, stop=True)
                muls_sb = ep.tile([128, 2 * OUT_C], F32, tag="muls")
                nc.vector.tensor_tensor(out=muls_sb[:], in0=o_ps[:], in1=bmuls_sb[:],
                                        op=mybir.AluOpType.add)
                rb = rows_of(b)
                nc.sync.dma_start(out=mu_out.ap()[128 * b:128 * b + rb, :],
                                  in_=muls_sb[:rb, 0:OUT_C])
                nc.sync.dma_start(out=ls_out.ap()[128 * b:128 * b + rb, :],
                                  in_=muls_sb[:rb, OUT_C:2 * OUT_C])

            for _ in range(reps):
                h_full = first_half()
                aggregation(h_full, epi2)

    nc.compile()
    return nc


TUNED_CFG = {"gbufs": 6, "sbufs": 6}


def build_in_maps(x, W1, b1, W_mu, b_mu, W_ls, b_ls, dinv, per_core):
    x = np.asarray(x)
    iota = np.tile(np.arange(128, dtype=np.float32), (128, 1)).astype(BF)
    ident = np.eye(128, dtype=np.float32).astype(BF)
    w1 = np.asarray(W1, np.float32).astype(BF)
    wmu = np.asarray(W_mu, np.float32).astype(BF)
    wls = np.asarray(W_ls, np.float32).astype(BF)
    b1t = np.tile(np.asarray(b1, np.float32), (128, 1))
    bmuls = np.tile(np.concatenate([np.asarray(b_mu, np.float32),
                                    np.asarray(b_ls, np.float32)]), (128, 1))
    XCOLS = NBLK * 128

    # replicated full x in table (chunk-major, assigned-position) order
    CB = CHUNK // 128 + 1
    XF = NU * CB * 128
    pa = np.stack([per_core[k][2] for k in range(N_CORES)])   # [cores, R]
    t = np.arange(N_NODES)
    jj, rem = t // CHUNK, t % CHUNK
    ks, ii = rem // UR, rem % UR
    node = ks * R + pa[ks, jj * UR + ii]
    xfull = x[node].astype(np.float32)
    dvfull = np.asarray(dinv, np.float32)[node]
    xTf = np.zeros((128, XF), dtype=BF)
    dinvf = np.ones((128, XF // 128), dtype=np.float32)
    for j in range(NU):
        seg = xfull[j * CHUNK:(j + 1) * CHUNK]
        xTf[:, CB * 128 * j:CB * 128 * j + CHUNK] = \
            np.ascontiguousarray(seg.T).astype(BF)
        pad = np.ones(CB * 128, np.float32)
        pad[:CHUNK] = dvfull[j * CHUNK:(j + 1) * CHUNK]
        dinvf[:, CB * j:CB * (j + 1)] = pad.reshape(CB, 128).T

    in_maps = []
    for k in range(N_CORES):
        idx16, dst128, perm = per_core[k]
        xk = x[R * k:R * (k + 1)][perm].astype(np.float32)
        xTk = np.zeros((128, XCOLS), dtype=BF)
        xTk[:, :R] = np.ascontiguousarray(xk.T).astype(BF)
        dv = dinv[R * k:R * (k + 1)][perm]
        padded = np.ones(NBLK * 128, dtype=np.float32)
        padded[:R] = dv
        dinv_blk = np.ascontiguousarray(padded.reshape(NBLK, 128).T)
        in_maps.append({
            "xT": xTk, "xTf": xTf, "dinvf_in": dinvf,
            "idx_in": idx16, "dst_in": dst128, "dinv_in": dinv_blk,
            "iota_in": iota, "ident_in": ident, "w1_in": w1, "wmu_in": wmu,
            "wls_in": wls, "b1_in": b1t, "bmuls_in": bmuls,
        })
    return in_maps


def kernel(x, edge_index, W1, b1, W_mu, b_mu, W_ls, b_ls):
    dinv, plan, per_core = _prep(np.asarray(edge_index))
    nc = _build(plan, cfg=TUNED_CFG)
    in_maps = build_in_maps(x, W1, b1, W_mu, b_mu, W_ls, b_ls, dinv, per_core)
    res = bass_utils.run_bass_kernel_spmd(nc, in_maps, core_ids=list(range(N_CORES)))
    mu = np.empty((N_NODES, OUT_C), dtype=np.float32)
    ls = np.empty((N_NODES, OUT_C), dtype=np.float32)
    for k in range(N_CORES):
        perm = per_core[k][2]
        mu[R * k + perm] = res.results[k]["mu_out"]
        ls[R * k + perm] = res.results[k]["ls_out"]
    return (mu, ls)



# revision 26
# speedup vs baseline: 1.4057x; 1.0388x over previous
"""GCN encoder (3x GCNConv: shared aggregation for mu/logstd) on 8 TRN2 NeuronCores.

Math: gcn_conv(x, A, W, b) = D^-1/2 (A+I) D^-1/2 (x W) + b, and the aggregation
commutes with the right matmul, so:
    y~ = dinv * (x @ W1)              (per-node row scale)
    h  = relu(dinv * SUM_edges y~[src] + b1)    (edge list includes self-loops)
    h~ = dinv * h
    g2 = dinv * SUM_edges h~[src]
    mu = g2 @ W_mu + b_mu ; logstd = g2 @ W_ls + b_ls

Sharding: nodes split contiguously across 8 cores (12500 each). Each core owns
the aggregation for its node range (dst-sharded). Gathered source rows come
from an AllGather'ed full y~ / h~ table (bf16), fetched with dma_gather using
int16 indices; the node space is split into 4 chunks of 25000 rows (both the
int16 range limit and the AllGather pipelining unit). Scatter-add is done by
one-hot matmuls accumulating in PSUM (S[e,d] = (dst_local[e]==d)).
"""
import numpy as np
import ml_dtypes
import concourse.bacc as bacc
import concourse.tile as tile
import concourse.bass as bass
import concourse.mybir as mybir
import concourse.bass_utils as bass_utils

N_CORES = 8
N_NODES = 100000
IN_C = 128
HID = 128
OUT_C = 64
R = N_NODES // N_CORES          # 12500 rows per core
NU = 4                          # src chunks / AllGather units
UR = R // NU                    # 3125 rows per unit per core
CHUNK = N_NODES // NU           # 25000 rows per (permuted) chunk
NBLK = (R + 127) // 128         # 98 dst blocks per core
BLK_GROUP = 16                  # dst blocks per PSUM group
GCOLS = 8                       # max 128-edge cols per dma_gather (1024 idxs = HW cap)

F32 = mybir.dt.float32
BF16 = mybir.dt.bfloat16
I16 = mybir.dt.int16
BF = ml_dtypes.bfloat16


def _wrap16(idx):
    """int16 indices -> [128, n/16] layout (16-partition wrap, replicated 8x)."""
    n = idx.shape[0]
    a = idx.astype(np.int16).reshape(n // 16, 16).T
    return np.ascontiguousarray(np.tile(a, (8, 1)))


def _prep(edge_index):
    """Host-side sharding prep: per-core padded edge streams + shared layout plan."""
    src = np.asarray(edge_index[0], dtype=np.int64)
    dst = np.asarray(edge_index[1], dtype=np.int64)
    # self-loops are NOT placed in the edge stream: their contribution
    # (dinv[d] * row[d]) is added in the epilogues from SBUF-local rows.
    # They still count toward the degree.
    deg = (np.bincount(dst, minlength=N_NODES) + 1).astype(np.float64)
    dinv = (1.0 / np.sqrt(deg)).astype(np.float32)

    # --- balanced node->position assignment -------------------------------
    # Reassign each node's position within its core (permutation within each
    # 3125-row AllGather unit, so source-chunk membership is unchanged) to
    # equalize the per-(block, chunk) edge counts: most segments then pack
    # into exactly ceil(mean/128) gather columns instead of paying the
    # max-over-cores Binomial tail.
    j_src = (src % R) // UR                     # source chunk (stable)
    cprof = np.bincount(dst * NU + j_src, minlength=N_NODES * NU)\
        .reshape(N_NODES, NU).astype(np.int64)  # per-dst-node chunk profile
    pos_of = np.empty(N_NODES, dtype=np.int64)  # node -> assigned local pos
    perm_of = []                                # per core: pos -> local node
    for k in range(N_CORES):
        pos_k = np.empty(R, dtype=np.int64)
        for u in range(NU):
            nodes = k * R + u * UR + np.arange(UR)
            P = cprof[nodes]                    # [UR, NU]
            lo, hi = u * UR, (u + 1) * UR
            b0, b1 = lo // 128, (hi + 127) // 128
            bins = [(max(128 * b, lo), min(128 * b + 128, hi))
                    for b in range(b0, b1)]
            cap = np.array([e - s for s, e in bins])
            # shared overflow blocks (cap 768 per cell) absorb each core's
            # Binomial excess so normal cells stay under 512 (w=4)
            ccap = np.array([768.0 if b % 24 == 23 else 512.0
                             for b in range(b0, b1)])
            fill = np.zeros(len(bins), dtype=np.int64)
            sums = np.zeros((len(bins), NU), dtype=np.int64)
            order = np.argsort(-P.sum(1), kind="stable")
            slot = np.empty(UR, dtype=np.int64)
            for i in order:
                ns = sums + P[i]
                hard = ns.max(axis=1) > ccap
                cost = (ns / ccap[:, None]).max(axis=1)
                cost[fill >= cap] = np.inf
                cost2 = np.where(hard, np.inf, cost)
                bsel = int(np.argmin(cost2))
                if not np.isfinite(cost2[bsel]):
                    bsel = int(np.argmin(cost))   # fallback: least overflow
                slot[i] = bins[bsel][0] + fill[bsel]
                fill[bsel] += 1
                sums[bsel] += P[i]
            pos_k[u * UR + np.arange(UR)] = slot
        pos_of[k * R:(k + 1) * R] = pos_k
        pk = np.empty(R, dtype=np.int64)
        pk[pos_k] = np.arange(R)
        perm_of.append(pk)                      # pos -> original local node

    # permuted (AllGather-major) source ids using ASSIGNED positions:
    # node at (core k, pos r=(j,i)) -> table row 25000j + 3125k + i
    k_of = src // R
    r_of = pos_of[src]
    j_of = r_of // UR
    i_of = r_of % UR
    psrc = CHUNK * j_of + UR * k_of + i_of
    c_of = psrc // CHUNK          # src chunk
    ci_of = psrc % CHUNK          # index within chunk (int16-safe, < 25000)

    kd = dst // R                 # owning core
    ld = pos_of[dst]
    b_of = ld // 128              # dst block
    dloc = ld % 128               # dst id within block

    g_of = b_of // BLK_GROUP      # block group
    # stream order: (core, group, chunk, block)
    order_key = ((kd * (NBLK // BLK_GROUP + 1) + g_of) * NU + c_of) * NBLK + b_of
    order = np.argsort(order_key, kind="stable")
    src_s, c_s, ci_s, kd_s, b_s, dloc_s = (
        a[order] for a in (src, c_of, ci_of, kd, b_of, dloc))

    # counts per (core, block, chunk) -> shared padded width w[b,c] (cols of 128)
    cnt = np.zeros((N_CORES, NBLK, NU), dtype=np.int64)
    np.add.at(cnt, (kd_s, b_s, c_s), 1)
    wmax = cnt.max(axis=0)                          # [NBLK, NU]
    w = ((wmax + 127) // 128).astype(np.int64)      # ceil; 0 stays 0

    # layout plan (shared across cores)
    ngroups = (NBLK + BLK_GROUP - 1) // BLK_GROUP
    col = 0
    seg = []          # (g, c, b, col_start, w_bc)
    for g in range(ngroups):
        blocks = range(g * BLK_GROUP, min((g + 1) * BLK_GROUP, NBLK))
        for c in range(NU):
            for b in blocks:
                if w[b, c] > 0:
                    seg.append((g, c, b, col, int(w[b, c])))
                    col += int(w[b, c])
    LT = col                                         # total 128-edge columns
    L = LT * 128

    # per-block first/last column (for PSUM start/stop flags)
    first_col = {}
    last_col = {}
    for (_g, _c, b, c0, wb) in seg:
        if b not in first_col:
            first_col[b] = c0
        last_col[b] = c0 + wb - 1
    block_of_col = np.full(LT, -1, dtype=np.int64)
    for (_g, _c, b, c0, wb) in seg:
        block_of_col[c0:c0 + wb] = b

    # gather calls: per (g, c) contiguous col range, split into <= GCOLS pieces
    calls = []        # (c, col_start, ncols)
    i = 0
    while i < len(seg):
        g, c = seg[i][0], seg[i][1]
        c0 = seg[i][3]
        cend = c0
        while i < len(seg) and seg[i][0] == g and seg[i][1] == c:
            cend = seg[i][3] + seg[i][4]
            i += 1
        p = c0
        while p < cend:
            n = min(GCOLS, cend - p)
            calls.append((c, p, n))
            p += n

    # per-core streams
    per_core = []
    # index into sorted stream: per (core, block, chunk) slice
    key_sorted = ((kd_s * (NBLK // BLK_GROUP + 1) + (b_s // BLK_GROUP)) * NU + c_s) * NBLK + b_s
    # boundaries via searchsorted on the sort keys
    for k in range(N_CORES):
        idx_arr = np.zeros(L, dtype=np.int16)
        dst_arr = np.full(L, -1.0, dtype=np.float32)
        sel = kd_s == k
        ci_k = ci_s[sel]
        b_k = b_s[sel]
        c_k = c_s[sel]
        dl_k = dloc_s[sel]
        g_k = b_k // BLK_GROUP
        key_k = (g_k * NU + c_k) * NBLK + b_k
        # stream is already sorted by key within core
        bounds = np.searchsorted(key_k, [(g * NU + c) * NBLK + b for (g, c, b, _c0, _w) in seg] +
                                 [(g * NU + c) * NBLK + b + 1 for (g, c, b, _c0, _w) in seg])
        nseg = len(seg)
        for si, (_g, _c, _b, c0, wb) in enumerate(seg):
            lo, hi = bounds[si], bounds[nseg + si]
            n = hi - lo
            assert n <= wb * 128
            # sort by source index within the segment: monotone HBM
            # addresses give the gather DMA row-buffer locality
            o = np.argsort(ci_k[lo:hi], kind="stable")
            idx_arr[c0 * 128: c0 * 128 + n] = ci_k[lo:hi][o]
            dst_arr[c0 * 128: c0 * 128 + n] = dl_k[lo:hi][o]
        # wrap idx into [128, L/16]; dst into [128, LT]
        idx16 = _wrap16(idx_arr)
        dst128 = np.ascontiguousarray(dst_arr.reshape(LT, 128).T.astype(BF))
        per_core.append((idx16, dst128, perm_of[k]))

    plan = dict(seg=seg, calls=calls, LT=LT, L=L,
                first_col=first_col, last_col=last_col, block_of_col=block_of_col)
    return dinv, plan, per_core


def _build(plan, reps=1, nocoll=False, cfg=None):
    """Build the SPMD Bass program (identical across cores).

    nocoll=True replaces collectives with local DMA copies (wrong values,
    same local-work shape) so TimelineSim / no-collective timing works.
    cfg: dict of tuning knobs (gbufs, sbufs, dense_gather, skip_s).
    """
    cfg = cfg or {}
    GBUFS = cfg.get("gbufs", 3)
    SBUFS = cfg.get("sbufs", 3)
    DENSE_GATHER = cfg.get("dense_gather", False)
    SKIP_S = cfg.get("skip_s", False)
    nc = bacc.Bacc("TRN2", target_bir_lowering=False, debug=False, num_devices=N_CORES,
                   num_swdge_queues=4)
    LT, L = plan["LT"], plan["L"]
    calls = plan["calls"]
    first_col, last_col = plan["first_col"], plan["last_col"]
    block_of_col = plan["block_of_col"]
    XCOLS = NBLK * 128  # zero-padded xT columns

    XF = NU * (CHUNK // 128 + 1) * 128  # full x, table order, per-chunk pad
    # inputs
    xT = nc.dram_tensor("xT", [128, XCOLS], BF16, kind="ExternalInput")
    xTf = nc.dram_tensor("xTf", [128, XF], BF16, kind="ExternalInput")
    dinvf_in = nc.dram_tensor("dinvf_in", [128, XF // 128], F32,
                              kind="ExternalInput")
    idx_in = nc.dram_tensor("idx_in", [128, L // 16], I16, kind="ExternalInput")
    dst_in = nc.dram_tensor("dst_in", [128, LT], BF16, kind="ExternalInput")
    dinv_in = nc.dram_tensor("dinv_in", [128, NBLK], F32, kind="ExternalInput")
    iota_in = nc.dram_tensor("iota_in", [128, 128], BF16, kind="ExternalInput")
    ident_in = nc.dram_tensor("ident_in", [128, 128], BF16, kind="ExternalInput")
    w1_in = nc.dram_tensor("w1_in", [128, HID], BF16, kind="ExternalInput")
    wmu_in = nc.dram_tensor("wmu_in", [HID, OUT_C], BF16, kind="ExternalInput")
    wls_in = nc.dram_tensor("wls_in", [HID, OUT_C], BF16, kind="ExternalInput")
    b1_in = nc.dram_tensor("b1_in", [128, HID], F32, kind="ExternalInput")
    bmuls_in = nc.dram_tensor("bmuls_in", [128, 2 * OUT_C], F32, kind="ExternalInput")
    # outputs
    mu_out = nc.dram_tensor("mu_out", [R, OUT_C], F32, kind="ExternalOutput")
    ls_out = nc.dram_tensor("ls_out", [R, OUT_C], F32, kind="ExternalOutput")

    def rows_of(b):
        return min(128, R - 128 * b)

    with tile.TileContext(nc) as tc:
        with (
            tc.tile_pool(name="const", bufs=1) as cpool,
            tc.tile_pool(name="xt", bufs=3) as xtp,
            tc.tile_pool(name="yh", bufs=4) as yhp,
            tc.tile_pool(name="gat", bufs=GBUFS) as gp,
            tc.tile_pool(name="sel", bufs=SBUFS) as sp,
            tc.tile_pool(name="epi", bufs=4) as ep,
            tc.tile_pool(name="psA", bufs=1, space="PSUM") as psA,
            tc.tile_pool(name="psB", bufs=2, space="PSUM") as psB,
            tc.tile_pool(name="dram", bufs=1, space="DRAM") as dram,
        ):
            # constants
            idx_sb = cpool.tile([128, L // 16], I16)
            dst_sb = cpool.tile([128, LT], BF16)
            dinv_sb = cpool.tile([128, NBLK], F32)
            iota_sb = cpool.tile([128, 128], BF16)
            ident_sb = cpool.tile([128, 128], BF16)
            w1_sb = cpool.tile([128, HID], BF16)
            wmu_sb = cpool.tile([HID, OUT_C], BF16)
            wls_sb = cpool.tile([HID, OUT_C], BF16)
            b1_sb = cpool.tile([128, HID], F32)
            bmuls_sb = cpool.tile([128, 2 * OUT_C], F32)
            # persistent local y~ / h~ rows (for the self-loop term)
            ylocal = cpool.tile([128, NBLK * HID], BF16)
            hlocal = cpool.tile([128, NBLK * HID], BF16)
            dinvf_sb = cpool.tile([128, XF // 128], F32)
            for sb, dr in ((idx_sb, idx_in), (dst_sb, dst_in), (dinv_sb, dinv_in),
                           (iota_sb, iota_in), (ident_sb, ident_in), (w1_sb, w1_in),
                           (wmu_sb, wmu_in), (wls_sb, wls_in), (b1_sb, b1_in),
                           (bmuls_sb, bmuls_in), (dinvf_sb, dinvf_in)):
                nc.sync.dma_start(out=sb[:], in_=dr.ap()[:])

            # internal DRAM
            h_in = dram.tile([R, HID], BF16)

            def alloc_full(pfx, shared=True):
                kw = dict(addr_space="Shared") if shared else {}
                return [dram.tile([CHUNK, HID], BF16, tag=f"{pfx}{j}",
                                  name=f"{pfx}{j}", **kw)
                        for j in range(NU)]

            # ---- phase 1a: local y~ rows (self-loop term only) ----
            def phase1_local():
                for b in range(NBLK):
                    xt_sb = xtp.tile([128, 128], BF16, tag="xt", name="xt_sb")
                    nc.sync.dma_start(out=xt_sb[:],
                                      in_=xT.ap()[:, 128 * b:128 * (b + 1)])
                    y_ps = psB.tile([128, HID], F32, tag="outps", space="PSUM",
                                    name="y_ps")
                    nc.tensor.matmul(out=y_ps[:], lhsT=xt_sb[:], rhs=w1_sb[:],
                                     start=True, stop=True)
                    nc.scalar.activation(out=ylocal[:, HID * b:HID * (b + 1)],
                                         in_=y_ps[:],
                                         func=mybir.ActivationFunctionType.Copy,
                                         scale=dinv_sb[:, b:b + 1])

            # ---- phase 1b: replicated full y~ table (replaces AllGather) ----
            # slab-batched DMA: 512KB loads/writes, not per-128-row transfers
            CB = CHUNK // 128 + 1        # 196 column-blocks per chunk
            SLAB = 16
            def phase1_full(y_full):
                for j in range(NU):
                    for s0 in range(0, CB, SLAB):
                        sn = min(SLAB, CB - s0)
                        xs = xtp.tile([128, SLAB * 128], BF16, tag="xs",
                                      name="xs")
                        nc.sync.dma_start(
                            out=xs[:, :sn * 128],
                            in_=xTf.ap()[:, 128 * (CB * j + s0):
                                         128 * (CB * j + s0 + sn)])
                        ys = yhp.tile([128, SLAB * HID], BF16, tag="ys",
                                      name="ys")
                        for q in range(sn):
                            tb = CB * j + s0 + q
                            y_ps = psB.tile([128, HID], F32, tag="outps",
                                            space="PSUM", name="y_ps")
                            nc.tensor.matmul(out=y_ps[:],
                                             lhsT=xs[:, 128 * q:128 * (q + 1)],
                                             rhs=w1_sb[:], start=True, stop=True)
                            nc.scalar.activation(
                                out=ys[:, HID * q:HID * (q + 1)], in_=y_ps[:],
                                func=mybir.ActivationFunctionType.Copy,
                                scale=dinvf_sb[:, tb:tb + 1])
                        rows = min(128 * sn, CHUNK - 128 * s0)
                        fw = rows // 128
                        if fw:
                            ysap = ys[:]
                            nc.sync.dma_start(
                                out=y_full[j][128 * s0:128 * s0 + 128 * fw, :]
                                    .rearrange("(w p) f -> p w f", p=128),
                                in_=bass.AP(ysap.tensor, ysap.offset,
                                            [ysap.ap[0], [HID, fw], [1, HID]]))
                        tail = rows - 128 * fw
                        if tail:
                            nc.sync.dma_start(
                                out=y_full[j][128 * s0 + 128 * fw:
                                              128 * s0 + rows, :],
                                in_=ys[:tail, HID * fw:HID * (fw + 1)])

            def agather(t_in, t_full):
                for j in range(NU):
                    if nocoll:
                        nc.sync.dma_start(out=t_full[j][0:UR, :],
                                          in_=t_in[UR * j:UR * (j + 1), :])
                    else:
                        nc.gpsimd.collective_compute(
                            "AllGather", mybir.AluOpType.bypass,
                            replica_groups=[list(range(N_CORES))],
                            ins=[t_in[UR * j:UR * (j + 1), :]],
                            outs=[t_full[j][:]],
                        )

            # per (group, bank): member blocks + first/last stream column.
            # PSUM start/stop must be bank-granular: start marks the whole 2KB
            # zero-region pending, so exactly one start and one stop per bank.
            ngroups = (NBLK + BLK_GROUP - 1) // BLK_GROUP
            bank_blocks = {}
            bank_first = {}
            bank_last = {}
            for b in range(NBLK):
                g, q = b // BLK_GROUP, (b % BLK_GROUP) // 4
                bank_blocks.setdefault((g, q), []).append(b)
                bank_first[(g, q)] = min(bank_first.get((g, q), 10 ** 9), first_col[b])
                bank_last[(g, q)] = max(bank_last.get((g, q), -1), last_col[b])
            def aggregation(table, epilogue):
                """Gather+one-hot-matmul aggregation over the shared edge stream."""
                acc = {}      # bank slot q -> psum tile
                done_blocks = set()
                for ci, (c, c0, ncols) in enumerate(calls):
                    g_tile = gp.tile([128, GCOLS, 128], BF16, tag="g")
                    if DENSE_GATHER:
                        nc.gpsimd.dma_start(
                            out=g_tile[:, 0:ncols, :],
                            in_=table[c][(c0 % 64) * 128:(c0 % 64) * 128 + 128 * ncols,
                                         :].rearrange("(w p) f -> p w f", p=128))
                    else:
                        nc.gpsimd.dma_gather(
                            out_ap=g_tile[:, 0:ncols, :],
                            in_ap=table[c][:],
                            idxs_ap=idx_sb[:, 8 * c0: 8 * (c0 + ncols)],
                            num_idxs=128 * ncols, num_idxs_reg=128 * ncols,
                            elem_size=128,
                            queue_num=ci % 4,
                            single_packet=cfg.get("single_packet", True),
                        )
                    s_tile = sp.tile([128, GCOLS, 128], BF16, tag="s")
                    if not SKIP_S:
                        dstap = dst_sb[:, c0:c0 + ncols].to_broadcast([128, ncols, 128])
                        iap = iota_sb[:]
                        iota_b = bass.AP(iap.tensor, iap.offset,
                                         [iap.ap[0], [0, ncols], iap.ap[1]])
                        nc.vector.tensor_tensor(out=s_tile[:, 0:ncols, :], in0=dstap,
                                                in1=iota_b, op=mybir.AluOpType.is_equal)
                    for t in range(ncols):
                        col = c0 + t
                        b = int(block_of_col[col])
                        g, q = b // BLK_GROUP, (b % BLK_GROUP) // 4
                        if col == bank_first[(g, q)]:
                            acc[q] = psA.tile([128, 512], F32, tag=f"acc{q}",
                                              name=f"acc{q}", space="PSUM")
                        a_ps = acc[q]
                        sl = slice(128 * (b % 4), 128 * (b % 4) + 128)
                        nc.tensor.matmul(out=a_ps[:, sl],
                                         lhsT=s_tile[:, t, :], rhs=g_tile[:, t, :],
                                         start=(col == bank_first[(g, q)]),
                                         stop=(col == bank_last[(g, q)]))
                        if col == bank_last[(g, q)]:
                            for bb in bank_blocks[(g, q)]:
                                sl2 = slice(128 * (bb % 4), 128 * (bb % 4) + 128)
                                epilogue(bb, a_ps[:, sl2])
                                done_blocks.add(bb)
                assert len(done_blocks) == NBLK

            # ---- agg1 epilogue: h~ = dinv * relu(dinv*(acc + y~) + b1) ----
            def epi1(b, acc_ap):
                # self-loop contribution: dinv_d^2 (x@W1)[d] = dinv_d * y~[d],
                # folded as (acc + y~[d]) * dinv_d
                t1 = ep.tile([128, HID], F32, tag="t1")
                nc.vector.tensor_tensor(out=t1[:], in0=acc_ap,
                                        in1=ylocal[:, HID * b:HID * (b + 1)],
                                        op=mybir.AluOpType.add)
                nc.vector.tensor_scalar_mul(out=t1[:], in0=t1[:],
                                            scalar1=dinv_sb[:, b:b + 1])
                nc.vector.tensor_tensor(out=t1[:], in0=t1[:], in1=b1_sb[:],
                                        op=mybir.AluOpType.add)
                h_sb = hlocal[:, HID * b:HID * (b + 1)]
                nc.scalar.activation(out=h_sb, in_=t1[:],
                                     func=mybir.ActivationFunctionType.Relu,
                                     scale=dinv_sb[:, b:b + 1])
                rb = rows_of(b)
                nc.sync.dma_start(out=h_in[128 * b:128 * b + rb, :], in_=h_sb[:rb, :])

            def first_half():
                phase1_local()
                y_full = alloc_full("y_full", shared=False)
                phase1_full(y_full)
                aggregation(y_full, epi1)
                h_full = alloc_full("h_full")
                agather(h_in, h_full)
                return h_full

            # ---- agg2 epilogue: g2 = dinv*(acc + dinv*h); mu/ls = g2 @ W + b ----
            def epi2(b, acc_ap):
                t2 = ep.tile([128, HID], F32, tag="t2")
                nc.vector.tensor_tensor(out=t2[:], in0=acc_ap,
                                        in1=hlocal[:, HID * b:HID * (b + 1)],
                                        op=mybir.AluOpType.add)
                g2_sb = ep.tile([128, HID], BF16, tag="g2")
                nc.scalar.activation(out=g2_sb[:], in_=t2[:],
                                     func=mybir.ActivationFunctionType.Copy,
                                     scale=dinv_sb[:, b:b + 1])
                tp_ps = psB.tile([128, HID], BF16, tag="tp", space="PSUM")
                nc.tensor.transpose(out=tp_ps[:], in_=g2_sb[:], identity=ident_sb[:])
                g2t_sb = ep.tile([128, HID], BF16, tag="g2t")
                nc.scalar.activation(out=g2t_sb[:], in_=tp_ps[:],
                                     func=mybir.ActivationFunctionType.Copy)
                o_ps = psB.tile([128, 2 * OUT_C], F32, tag="outps", space="PSUM")
                nc.tensor.matmul(out=o_ps[:, 0:OUT_C], lhsT=g2t_sb[:], rhs=wmu_sb[:],
                                 start=True, stop=True)
                nc.tensor.matmul(out=o_ps[:, OUT_C:2 * OUT_C], lhsT=g2t_sb[:],
                                 rhs=wls_sb[:], start=True, stop=True)
                muls_sb = ep.tile([128, 2 * OUT_C], F32, tag="muls")
                nc.vector.tensor_tensor(out=muls_sb[:], in0=o_ps[:], in1=bmuls_sb[:],
                                        op=mybir.AluOpType.add)
                rb = rows_of(b)
                nc.sync.dma_start(out=mu_out.ap()[128 * b:128 * b + rb, :],
                                  in_=muls_sb[:rb, 0:OUT_C])
                nc.sync.dma_start(out=ls_out.ap()[128 * b:128 * b + rb, :],
                                  in_=muls_sb[:rb, OUT_C:2 * OUT_C])

            for _ in range(reps):
                h_full = first_half()
                aggregation(h_full, epi2)

    nc.compile()
    return nc


TUNED_CFG = {"gbufs": 10, "sbufs": 10}


def build_in_maps(x, W1, b1, W_mu, b_mu, W_ls, b_ls, dinv, per_core):
    x = np.asarray(x)
    iota = np.tile(np.arange(128, dtype=np.float32), (128, 1)).astype(BF)
    ident = np.eye(128, dtype=np.float32).astype(BF)
    w1 = np.asarray(W1, np.float32).astype(BF)
    wmu = np.asarray(W_mu, np.float32).astype(BF)
    wls = np.asarray(W_ls, np.float32).astype(BF)
    b1t = np.tile(np.asarray(b1, np.float32), (128, 1))
    bmuls = np.tile(np.concatenate([np.asarray(b_mu, np.float32),
                                    np.asarray(b_ls, np.float32)]), (128, 1))
    XCOLS = NBLK * 128

    # replicated full x in table (chunk-major, assigned-position) order
    CB = CHUNK // 128 + 1
    XF = NU * CB * 128
    pa = np.stack([per_core[k][2] for k in range(N_CORES)])   # [cores, R]
    t = np.arange(N_NODES)
    jj, rem = t // CHUNK, t % CHUNK
    ks, ii = rem // UR, rem % UR
    node = ks * R + pa[ks, jj * UR + ii]
    xfull = x[node].astype(np.float32)
    dvfull = np.asarray(dinv, np.float32)[node]
    xTf = np.zeros((128, XF), dtype=BF)
    dinvf = np.ones((128, XF // 128), dtype=np.float32)
    for j in range(NU):
        seg = xfull[j * CHUNK:(j + 1) * CHUNK]
        xTf[:, CB * 128 * j:CB * 128 * j + CHUNK] = \
            np.ascontiguousarray(seg.T).astype(BF)
        pad = np.ones(CB * 128, np.float32)
        pad[:CHUNK] = dvfull[j * CHUNK:(j + 1) * CHUNK]
        dinvf[:, CB * j:CB * (j + 1)] = pad.reshape(CB, 128).T

    in_maps = []
    for k in range(N_CORES):
        idx16, dst128, perm = per_core[k]
        xk = x[R * k:R * (k + 1)][perm].astype(np.float32)
        xTk = np.zeros((128, XCOLS), dtype=BF)
        xTk[:, :R] = np.ascontiguousarray(xk.T).astype(BF)
        dv = dinv[R * k:R * (k + 1)][perm]
        padded = np.ones(NBLK * 128, dtype=np.float32)
        padded[:R] = dv
        dinv_blk = np.ascontiguousarray(padded.reshape(NBLK, 128).T)
        in_maps.append({
            "xT": xTk, "xTf": xTf, "dinvf_in": dinvf,
            "idx_in": idx16, "dst_in": dst128, "dinv_in": dinv_blk,
            "iota_in": iota, "ident_in": ident, "w1_in": w1, "wmu_in": wmu,
            "wls_in": wls, "b1_in": b1t, "bmuls_in": bmuls,
        })
    return in_maps


def kernel(x, edge_index, W1, b1, W_mu, b_mu, W_ls, b_ls):
    dinv, plan, per_core = _prep(np.asarray(edge_index))
    nc = _build(plan, cfg=TUNED_CFG)
    in_maps = build_in_maps(x, W1, b1, W_mu, b_mu, W_ls, b_ls, dinv, per_core)
    res = bass_utils.run_bass_kernel_spmd(nc, in_maps, core_ids=list(range(N_CORES)))
    mu = np.empty((N_NODES, OUT_C), dtype=np.float32)
    ls = np.empty((N_NODES, OUT_C), dtype=np.float32)
    for k in range(N_CORES):
        perm = per_core[k][2]
        mu[R * k + perm] = res.results[k]["mu_out"]
        ls[R * k + perm] = res.results[k]["ls_out"]
    return (mu, ls)



# revision 36
# speedup vs baseline: 1.4858x; 1.0569x over previous
"""GCN encoder (3x GCNConv: shared aggregation for mu/logstd) on 8 TRN2 NeuronCores.

Math: gcn_conv(x, A, W, b) = D^-1/2 (A+I) D^-1/2 (x W) + b, and the aggregation
commutes with the right matmul, so:
    y~ = dinv * (x @ W1)              (per-node row scale)
    h  = relu(dinv * (SUM_edges y~[src] + y~[dst]) + b1)
    h~ = dinv * h
    g2 = dinv * (SUM_edges h~[src] + h~[dst])
    mu = g2 @ W_mu + b_mu ; logstd = g2 @ W_ls + b_ls
(self-loops are lifted out of the edge stream: their contribution is the
local +row[dst] term added in the epilogues from SBUF-resident rows)

Sharding: nodes split contiguously across 8 cores (12500 each). Each core owns
the aggregation for its node range (dst-sharded). The full y~ table is
computed REPLICATED on every core (x is a full input; 784 extra 128x128
matmuls are ~free) so only the h~ table needs an AllGather. Gathered source
rows are fetched with dma_gather (1024 int16 idxs/call = HW cap, 256B rows);
the node space is split into 4 chunks of 25000 rows (int16 range limit and
AllGather pipelining unit). Scatter-add is done by one-hot matmuls
accumulating in PSUM (S[e,d] = (dst_local[e]==d)).

Node positions are reassigned within each 3125-row AllGather unit (host-side
permutation, outputs unpermuted on host) to balance per-(block, chunk) edge
counts toward the 512-edge/4-column packing: the gathers are HBM-latency
bound, so time scales with padded descriptor count and with the max-over-core
segment widths that the shared plan must accommodate. Measured ~2.0ms vs the
2.87ms session baseline.
"""
import numpy as np
import ml_dtypes
import concourse.bacc as bacc
import concourse.tile as tile
import concourse.bass as bass
import concourse.mybir as mybir
import concourse.bass_utils as bass_utils

N_CORES = 8
N_NODES = 100000
IN_C = 128
HID = 128
OUT_C = 64
R = N_NODES // N_CORES          # 12500 rows per core
NU = 4                          # src chunks / AllGather units
UR = R // NU                    # 3125 rows per unit per core
CHUNK = N_NODES // NU           # 25000 rows per (permuted) chunk
NBLK = (R + 127) // 128         # 98 dst blocks per core
BLK_GROUP = 16                  # dst blocks per PSUM group
GCOLS = 8                       # max 128-edge cols per dma_gather (1024 idxs = HW cap)

F32 = mybir.dt.float32
BF16 = mybir.dt.bfloat16
I16 = mybir.dt.int16
BF = ml_dtypes.bfloat16


def _wrap16(idx):
    """int16 indices -> [128, n/16] layout (16-partition wrap, replicated 8x)."""
    n = idx.shape[0]
    a = idx.astype(np.int16).reshape(n // 16, 16).T
    return np.ascontiguousarray(np.tile(a, (8, 1)))


def _prep(edge_index):
    """Host-side sharding prep: per-core padded edge streams + shared layout plan."""
    src = np.asarray(edge_index[0], dtype=np.int64)
    dst = np.asarray(edge_index[1], dtype=np.int64)
    # self-loops are NOT placed in the edge stream: their contribution
    # (dinv[d] * row[d]) is added in the epilogues from SBUF-local rows.
    # They still count toward the degree.
    deg = (np.bincount(dst, minlength=N_NODES) + 1).astype(np.float64)
    dinv = (1.0 / np.sqrt(deg)).astype(np.float32)

    # --- balanced node->position assignment -------------------------------
    # Reassign each node's position within its core (permutation within each
    # 3125-row AllGather unit, so source-chunk membership is unchanged) to
    # equalize the per-(block, chunk) edge counts: most segments then pack
    # into exactly ceil(mean/128) gather columns instead of paying the
    # max-over-cores Binomial tail.
    j_src = (src % R) // UR                     # source chunk (stable)
    cprof = np.bincount(dst * NU + j_src, minlength=N_NODES * NU)\
        .reshape(N_NODES, NU).astype(np.int64)  # per-dst-node chunk profile
    pos_of = np.empty(N_NODES, dtype=np.int64)  # node -> assigned local pos
    perm_of = []                                # per core: pos -> local node
    for k in range(N_CORES):
        pos_k = np.empty(R, dtype=np.int64)
        for u in range(NU):
            nodes = k * R + u * UR + np.arange(UR)
            P = cprof[nodes]                    # [UR, NU]
            lo, hi = u * UR, (u + 1) * UR
            b0, b1 = lo // 128, (hi + 127) // 128
            bins = [(max(128 * b, lo), min(128 * b + 128, hi))
                    for b in range(b0, b1)]
            cap = np.array([e - s for s, e in bins])
            # shared overflow blocks (cap 640 per cell) absorb each core's
            # Binomial excess so normal cells stay under 512 (w=4)
            ccap = np.array([640.0 if b % 12 == 11 else 512.0
                             for b in range(b0, b1)])
            fill = np.zeros(len(bins), dtype=np.int64)
            sums = np.zeros((len(bins), NU), dtype=np.int64)
            order = np.argsort(-P.sum(1), kind="stable")
            slot = np.empty(UR, dtype=np.int64)
            for i in order:
                ns = sums + P[i]
                hard = ns.max(axis=1) > ccap
                cost = np.maximum((ns / ccap[:, None]).max(axis=1),
                                  0.98 * (fill + 1) / np.maximum(cap, 1))
                cost[fill >= cap] = np.inf
                cost2 = np.where(hard, np.inf, cost)
                bsel = int(np.argmin(cost2))
                if not np.isfinite(cost2[bsel]):
                    bsel = int(np.argmin(cost))   # fallback: least overflow
                slot[i] = bins[bsel][0] + fill[bsel]
                fill[bsel] += 1
                sums[bsel] += P[i]
            pos_k[u * UR + np.arange(UR)] = slot
        pos_of[k * R:(k + 1) * R] = pos_k
        pk = np.empty(R, dtype=np.int64)
        pk[pos_k] = np.arange(R)
        perm_of.append(pk)                      # pos -> original local node

    # permuted (AllGather-major) source ids using ASSIGNED positions:
    # node at (core k, pos r=(j,i)) -> table row 25000j + 3125k + i
    k_of = src // R
    r_of = pos_of[src]
    j_of = r_of // UR
    i_of = r_of % UR
    psrc = CHUNK * j_of + UR * k_of + i_of
    c_of = psrc // CHUNK          # src chunk
    ci_of = psrc % CHUNK          # index within chunk (int16-safe, < 25000)

    kd = dst // R                 # owning core
    ld = pos_of[dst]
    b_of = ld // 128              # dst block
    dloc = ld % 128               # dst id within block

    g_of = b_of // BLK_GROUP      # block group
    # stream order: (core, group, chunk, block)
    order_key = ((kd * (NBLK // BLK_GROUP + 1) + g_of) * NU + c_of) * NBLK + b_of
    order = np.argsort(order_key, kind="stable")
    src_s, c_s, ci_s, kd_s, b_s, dloc_s = (
        a[order] for a in (src, c_of, ci_of, kd, b_of, dloc))

    # counts per (core, block, chunk) -> shared padded width w[b,c] (cols of 128)
    cnt = np.zeros((N_CORES, NBLK, NU), dtype=np.int64)
    np.add.at(cnt, (kd_s, b_s, c_s), 1)
    wmax = cnt.max(axis=0)                          # [NBLK, NU]
    w = ((wmax + 127) // 128).astype(np.int64)      # ceil; 0 stays 0

    # layout plan (shared across cores)
    ngroups = (NBLK + BLK_GROUP - 1) // BLK_GROUP
    col = 0
    seg = []          # (g, c, b, col_start, w_bc)
    for g in range(ngroups):
        blocks = range(g * BLK_GROUP, min((g + 1) * BLK_GROUP, NBLK))
        for c in range(NU):
            for b in blocks:
                if w[b, c] > 0:
                    seg.append((g, c, b, col, int(w[b, c])))
                    col += int(w[b, c])
    LT = col                                         # total 128-edge columns
    L = LT * 128

    # per-block first/last column (for PSUM start/stop flags)
    first_col = {}
    last_col = {}
    for (_g, _c, b, c0, wb) in seg:
        if b not in first_col:
            first_col[b] = c0
        last_col[b] = c0 + wb - 1
    block_of_col = np.full(LT, -1, dtype=np.int64)
    for (_g, _c, b, c0, wb) in seg:
        block_of_col[c0:c0 + wb] = b

    # gather calls: per (g, c) contiguous col range, split into <= GCOLS pieces
    calls = []        # (c, col_start, ncols)
    i = 0
    while i < len(seg):
        g, c = seg[i][0], seg[i][1]
        c0 = seg[i][3]
        cend = c0
        while i < len(seg) and seg[i][0] == g and seg[i][1] == c:
            cend = seg[i][3] + seg[i][4]
            i += 1
        p = c0
        while p < cend:
            n = min(GCOLS, cend - p)
            calls.append((c, p, n))
            p += n

    # per-core streams
    per_core = []
    # index into sorted stream: per (core, block, chunk) slice
    key_sorted = ((kd_s * (NBLK // BLK_GROUP + 1) + (b_s // BLK_GROUP)) * NU + c_s) * NBLK + b_s
    # boundaries via searchsorted on the sort keys
    for k in range(N_CORES):
        idx_arr = np.zeros(L, dtype=np.int16)
        dst_arr = np.full(L, -1.0, dtype=np.float32)
        sel = kd_s == k
        ci_k = ci_s[sel]
        b_k = b_s[sel]
        c_k = c_s[sel]
        dl_k = dloc_s[sel]
        g_k = b_k // BLK_GROUP
        key_k = (g_k * NU + c_k) * NBLK + b_k
        # stream is already sorted by key within core
        bounds = np.searchsorted(key_k, [(g * NU + c) * NBLK + b for (g, c, b, _c0, _w) in seg] +
                                 [(g * NU + c) * NBLK + b + 1 for (g, c, b, _c0, _w) in seg])
        nseg = len(seg)
        for si, (_g, _c, _b, c0, wb) in enumerate(seg):
            lo, hi = bounds[si], bounds[nseg + si]
            n = hi - lo
            assert n <= wb * 128
            # sort by source index within the segment: monotone HBM
            # addresses give the gather DMA row-buffer locality
            o = np.argsort(ci_k[lo:hi], kind="stable")
            idx_arr[c0 * 128: c0 * 128 + n] = ci_k[lo:hi][o]
            dst_arr[c0 * 128: c0 * 128 + n] = dl_k[lo:hi][o]
        # wrap idx into [128, L/16]; dst into [128, LT]
        idx16 = _wrap16(idx_arr)
        dst128 = np.ascontiguousarray(dst_arr.reshape(LT, 128).T.astype(BF))
        per_core.append((idx16, dst128, perm_of[k]))

    plan = dict(seg=seg, calls=calls, LT=LT, L=L,
                first_col=first_col, last_col=last_col, block_of_col=block_of_col)
    return dinv, plan, per_core


def _build(plan, reps=1, nocoll=False, cfg=None):
    """Build the SPMD Bass program (identical across cores).

    nocoll=True replaces collectives with local DMA copies (wrong values,
    same local-work shape) so TimelineSim / no-collective timing works.
    cfg: dict of tuning knobs (gbufs, sbufs, dense_gather, skip_s).
    """
    cfg = cfg or {}
    GBUFS = cfg.get("gbufs", 3)
    SBUFS = cfg.get("sbufs", 3)
    DENSE_GATHER = cfg.get("dense_gather", False)
    SKIP_S = cfg.get("skip_s", False)
    nc = bacc.Bacc("TRN2", target_bir_lowering=False, debug=False, num_devices=N_CORES,
                   num_swdge_queues=4)
    LT, L = plan["LT"], plan["L"]
    calls = plan["calls"]
    first_col, last_col = plan["first_col"], plan["last_col"]
    block_of_col = plan["block_of_col"]
    XCOLS = NBLK * 128  # zero-padded xT columns

    XF = NU * (CHUNK // 128 + 1) * 128  # full x, table order, per-chunk pad
    # inputs
    xT = nc.dram_tensor("xT", [128, XCOLS], BF16, kind="ExternalInput")
    xTf = nc.dram_tensor("xTf", [128, XF], BF16, kind="ExternalInput")
    dinvf_in = nc.dram_tensor("dinvf_in", [128, XF // 128], F32,
                              kind="ExternalInput")
    idx_in = nc.dram_tensor("idx_in", [128, L // 16], I16, kind="ExternalInput")
    dst_in = nc.dram_tensor("dst_in", [128, LT], BF16, kind="ExternalInput")
    dinv_in = nc.dram_tensor("dinv_in", [128, NBLK], F32, kind="ExternalInput")
    iota_in = nc.dram_tensor("iota_in", [128, 128], BF16, kind="ExternalInput")
    ident_in = nc.dram_tensor("ident_in", [128, 128], BF16, kind="ExternalInput")
    w1_in = nc.dram_tensor("w1_in", [128, HID], BF16, kind="ExternalInput")
    wmu_in = nc.dram_tensor("wmu_in", [HID, OUT_C], BF16, kind="ExternalInput")
    wls_in = nc.dram_tensor("wls_in", [HID, OUT_C], BF16, kind="ExternalInput")
    b1_in = nc.dram_tensor("b1_in", [128, HID], F32, kind="ExternalInput")
    bmuls_in = nc.dram_tensor("bmuls_in", [128, 2 * OUT_C], F32, kind="ExternalInput")
    # outputs
    mu_out = nc.dram_tensor("mu_out", [R, OUT_C], F32, kind="ExternalOutput")
    ls_out = nc.dram_tensor("ls_out", [R, OUT_C], F32, kind="ExternalOutput")

    def rows_of(b):
        return min(128, R - 128 * b)

    with tile.TileContext(nc) as tc:
        with (
            tc.tile_pool(name="const", bufs=1) as cpool,
            tc.tile_pool(name="xt", bufs=3) as xtp,
            tc.tile_pool(name="yh", bufs=4) as yhp,
            tc.tile_pool(name="gat", bufs=GBUFS) as gp,
            tc.tile_pool(name="sel", bufs=SBUFS) as sp,
            tc.tile_pool(name="epi", bufs=4) as ep,
            tc.tile_pool(name="psA", bufs=1, space="PSUM") as psA,
            tc.tile_pool(name="psB", bufs=2, space="PSUM") as psB,
            tc.tile_pool(name="dram", bufs=1, space="DRAM") as dram,
        ):
            # constants
            idx_sb = cpool.tile([128, L // 16], I16)
            dst_sb = cpool.tile([128, LT], BF16)
            dinv_sb = cpool.tile([128, NBLK], F32)
            iota_sb = cpool.tile([128, 128], BF16)
            ident_sb = cpool.tile([128, 128], BF16)
            w1_sb = cpool.tile([128, HID], BF16)
            wmu_sb = cpool.tile([HID, OUT_C], BF16)
            wls_sb = cpool.tile([HID, OUT_C], BF16)
            b1_sb = cpool.tile([128, HID], F32)
            bmuls_sb = cpool.tile([128, 2 * OUT_C], F32)
            # persistent local y~ / h~ rows (for the self-loop term)
            ylocal = cpool.tile([128, NBLK * HID], BF16)
            hlocal = cpool.tile([128, NBLK * HID], BF16)
            dinvf_sb = cpool.tile([128, XF // 128], F32)
            for sb, dr in ((idx_sb, idx_in), (dst_sb, dst_in), (dinv_sb, dinv_in),
                           (iota_sb, iota_in), (ident_sb, ident_in), (w1_sb, w1_in),
                           (wmu_sb, wmu_in), (wls_sb, wls_in), (b1_sb, b1_in),
                           (bmuls_sb, bmuls_in), (dinvf_sb, dinvf_in)):
                nc.sync.dma_start(out=sb[:], in_=dr.ap()[:])

            # internal DRAM
            h_in = dram.tile([R, HID], BF16)

            def alloc_full(pfx, shared=True):
                kw = dict(addr_space="Shared") if shared else {}
                return [dram.tile([CHUNK, HID], BF16, tag=f"{pfx}{j}",
                                  name=f"{pfx}{j}", **kw)
                        for j in range(NU)]

            # ---- phase 1a: local y~ rows (self-loop term only) ----
            def phase1_local():
                for b in range(NBLK):
                    xt_sb = xtp.tile([128, 128], BF16, tag="xt", name="xt_sb")
                    nc.sync.dma_start(out=xt_sb[:],
                                      in_=xT.ap()[:, 128 * b:128 * (b + 1)])
                    y_ps = psB.tile([128, HID], F32, tag="outps", space="PSUM",
                                    name="y_ps")
                    nc.tensor.matmul(out=y_ps[:], lhsT=xt_sb[:], rhs=w1_sb[:],
                                     start=True, stop=True)
                    nc.scalar.activation(out=ylocal[:, HID * b:HID * (b + 1)],
                                         in_=y_ps[:],
                                         func=mybir.ActivationFunctionType.Copy,
                                         scale=dinv_sb[:, b:b + 1])

            # ---- phase 1b: replicated full y~ table (replaces AllGather) ----
            # slab-batched DMA: 512KB loads/writes, not per-128-row transfers
            CB = CHUNK // 128 + 1        # 196 column-blocks per chunk
            SLAB = 16
            def phase1_full(y_full):
                for j in range(NU):
                    for s0 in range(0, CB, SLAB):
                        sn = min(SLAB, CB - s0)
                        xs = xtp.tile([128, SLAB * 128], BF16, tag="xs",
                                      name="xs")
                        nc.sync.dma_start(
                            out=xs[:, :sn * 128],
                            in_=xTf.ap()[:, 128 * (CB * j + s0):
                                         128 * (CB * j + s0 + sn)])
                        ys = yhp.tile([128, SLAB * HID], BF16, tag="ys",
                                      name="ys")
                        for q in range(sn):
                            tb = CB * j + s0 + q
                            y_ps = psB.tile([128, HID], F32, tag="outps",
                                            space="PSUM", name="y_ps")
                            nc.tensor.matmul(out=y_ps[:],
                                             lhsT=xs[:, 128 * q:128 * (q + 1)],
                                             rhs=w1_sb[:], start=True, stop=True)
                            nc.scalar.activation(
                                out=ys[:, HID * q:HID * (q + 1)], in_=y_ps[:],
                                func=mybir.ActivationFunctionType.Copy,
                                scale=dinvf_sb[:, tb:tb + 1])
                        rows = min(128 * sn, CHUNK - 128 * s0)
                        fw = rows // 128
                        if fw:
                            ysap = ys[:]
                            nc.sync.dma_start(
                                out=y_full[j][128 * s0:128 * s0 + 128 * fw, :]
                                    .rearrange("(w p) f -> p w f", p=128),
                                in_=bass.AP(ysap.tensor, ysap.offset,
                                            [ysap.ap[0], [HID, fw], [1, HID]]))
                        tail = rows - 128 * fw
                        if tail:
                            nc.sync.dma_start(
                                out=y_full[j][128 * s0 + 128 * fw:
                                              128 * s0 + rows, :],
                                in_=ys[:tail, HID * fw:HID * (fw + 1)])

            def agather(t_in, t_full):
                for j in range(NU):
                    if nocoll:
                        nc.sync.dma_start(out=t_full[j][0:UR, :],
                                          in_=t_in[UR * j:UR * (j + 1), :])
                    else:
                        nc.gpsimd.collective_compute(
                            "AllGather", mybir.AluOpType.bypass,
                            replica_groups=[list(range(N_CORES))],
                            ins=[t_in[UR * j:UR * (j + 1), :]],
                            outs=[t_full[j][:]],
                        )

            # per (group, bank): member blocks + first/last stream column.
            # PSUM start/stop must be bank-granular: start marks the whole 2KB
            # zero-region pending, so exactly one start and one stop per bank.
            ngroups = (NBLK + BLK_GROUP - 1) // BLK_GROUP
            bank_blocks = {}
            bank_first = {}
            bank_last = {}
            for b in range(NBLK):
                g, q = b // BLK_GROUP, (b % BLK_GROUP) // 4
                bank_blocks.setdefault((g, q), []).append(b)
                bank_first[(g, q)] = min(bank_first.get((g, q), 10 ** 9), first_col[b])
                bank_last[(g, q)] = max(bank_last.get((g, q), -1), last_col[b])
            def aggregation(table, epilogue, bank_start=None, bank_flush=None):
                """Gather+one-hot-matmul aggregation over the shared edge stream."""
                acc = {}      # bank slot q -> psum tile
                done_blocks = set()
                for ci, (c, c0, ncols) in enumerate(calls):
                    g_tile = gp.tile([128, GCOLS, 128], BF16, tag="g")
                    if DENSE_GATHER:
                        nc.gpsimd.dma_start(
                            out=g_tile[:, 0:ncols, :],
                            in_=table[c][(c0 % 64) * 128:(c0 % 64) * 128 + 128 * ncols,
                                         :].rearrange("(w p) f -> p w f", p=128))
                    else:
                        nc.gpsimd.dma_gather(
                            out_ap=g_tile[:, 0:ncols, :],
                            in_ap=table[c][:],
                            idxs_ap=idx_sb[:, 8 * c0: 8 * (c0 + ncols)],
                            num_idxs=128 * ncols, num_idxs_reg=128 * ncols,
                            elem_size=128,
                            queue_num=ci % 4,
                            single_packet=cfg.get("single_packet", True),
                        )
                    s_tile = sp.tile([128, GCOLS, 128], BF16, tag="s")
                    if not SKIP_S:
                        dstap = dst_sb[:, c0:c0 + ncols].to_broadcast([128, ncols, 128])
                        iap = iota_sb[:]
                        iota_b = bass.AP(iap.tensor, iap.offset,
                                         [iap.ap[0], [0, ncols], iap.ap[1]])
                        nc.vector.tensor_tensor(out=s_tile[:, 0:ncols, :], in0=dstap,
                                                in1=iota_b, op=mybir.AluOpType.is_equal)
                    for t in range(ncols):
                        col = c0 + t
                        b = int(block_of_col[col])
                        g, q = b // BLK_GROUP, (b % BLK_GROUP) // 4
                        if col == bank_first[(g, q)]:
                            acc[q] = psA.tile([128, 512], F32, tag=f"acc{q}",
                                              name=f"acc{q}", space="PSUM")
                        a_ps = acc[q]
                        sl = slice(128 * (b % 4), 128 * (b % 4) + 128)
                        nc.tensor.matmul(out=a_ps[:, sl],
                                         lhsT=s_tile[:, t, :], rhs=g_tile[:, t, :],
                                         start=(col == bank_first[(g, q)]),
                                         stop=(col == bank_last[(g, q)]))
                        if col == bank_last[(g, q)]:
                            blocks = bank_blocks[(g, q)]
                            ctx = bank_start(blocks) if bank_start else None
                            for bi, bb in enumerate(blocks):
                                sl2 = slice(128 * (bb % 4), 128 * (bb % 4) + 128)
                                epilogue(bb, a_ps[:, sl2], ctx, bi)
                                done_blocks.add(bb)
                            if bank_flush:
                                bank_flush(blocks, ctx)
                assert len(done_blocks) == NBLK

            # ---- agg1 epilogue: h~ = dinv * relu(dinv*(acc + y~) + b1) ----
            def epi1(b, acc_ap, _ctx, _bi):
                # self-loop contribution: dinv_d^2 (x@W1)[d] = dinv_d * y~[d],
                # folded as (acc + y~[d]) * dinv_d
                t1 = ep.tile([128, HID], F32, tag="t1")
                nc.vector.tensor_tensor(out=t1[:], in0=acc_ap,
                                        in1=ylocal[:, HID * b:HID * (b + 1)],
                                        op=mybir.AluOpType.add)
                nc.vector.tensor_scalar_mul(out=t1[:], in0=t1[:],
                                            scalar1=dinv_sb[:, b:b + 1])
                nc.vector.tensor_tensor(out=t1[:], in0=t1[:], in1=b1_sb[:],
                                        op=mybir.AluOpType.add)
                nc.scalar.activation(out=hlocal[:, HID * b:HID * (b + 1)],
                                     in_=t1[:],
                                     func=mybir.ActivationFunctionType.Relu,
                                     scale=dinv_sb[:, b:b + 1])

            def flush1(blocks, _ctx):
                # one slab DMA per bank: hlocal[blocks] -> h_in rows
                b0, n = blocks[0], len(blocks)
                rows = min(128 * n, R - 128 * b0)
                fw = rows // 128
                hap = hlocal[:]
                if fw:
                    nc.sync.dma_start(
                        out=h_in[128 * b0:128 * b0 + 128 * fw, :]
                            .rearrange("(w p) f -> p w f", p=128),
                        in_=bass.AP(hap.tensor, hap.offset + HID * b0,
                                    [hap.ap[0], [HID, fw], [1, HID]]))
                tail = rows - 128 * fw
                if tail:
                    nc.sync.dma_start(
                        out=h_in[128 * b0 + 128 * fw:128 * b0 + rows, :],
                        in_=hlocal[:tail, HID * (b0 + fw):HID * (b0 + fw + 1)])

            def first_half():
                phase1_local()
                y_full = alloc_full("y_full", shared=False)
                phase1_full(y_full)
                aggregation(y_full, epi1, bank_flush=flush1)
                h_full = alloc_full("h_full")
                agather(h_in, h_full)
                return h_full

            # ---- agg2 epilogue: g2 = dinv*(acc + dinv*h); mu/ls = g2 @ W + b ----
            def bank2_start(blocks):
                mub = ep.tile([128, len(blocks) * 2 * OUT_C], F32, tag="muls",
                              name="mub")
                return mub

            def epi2(b, acc_ap, mub, bi):
                t2 = ep.tile([128, HID], F32, tag="t2")
                nc.vector.tensor_tensor(out=t2[:], in0=acc_ap,
                                        in1=hlocal[:, HID * b:HID * (b + 1)],
                                        op=mybir.AluOpType.add)
                g2_sb = ep.tile([128, HID], BF16, tag="g2")
                nc.scalar.activation(out=g2_sb[:], in_=t2[:],
                                     func=mybir.ActivationFunctionType.Copy,
                                     scale=dinv_sb[:, b:b + 1])
                tp_ps = psB.tile([128, HID], BF16, tag="tp", space="PSUM")
                nc.tensor.transpose(out=tp_ps[:], in_=g2_sb[:], identity=ident_sb[:])
                g2t_sb = ep.tile([128, HID], BF16, tag="g2t")
                nc.scalar.activation(out=g2t_sb[:], in_=tp_ps[:],
                                     func=mybir.ActivationFunctionType.Copy)
                o_ps = psB.tile([128, 2 * OUT_C], F32, tag="outps", space="PSUM")
                nc.tensor.matmul(out=o_ps[:, 0:OUT_C], lhsT=g2t_sb[:], rhs=wmu_sb[:],
                                 start=True, stop=True)
                nc.tensor.matmul(out=o_ps[:, OUT_C:2 * OUT_C], lhsT=g2t_sb[:],
                                 rhs=wls_sb[:], start=True, stop=True)
                nc.vector.tensor_tensor(
                    out=mub[:, 2 * OUT_C * bi:2 * OUT_C * (bi + 1)],
                    in0=o_ps[:], in1=bmuls_sb[:], op=mybir.AluOpType.add)

            def flush2(blocks, mub):
                # two slab DMAs per bank: mu / ls rows for the whole bank
                b0, n = blocks[0], len(blocks)
                rows = min(128 * n, R - 128 * b0)
                fw = rows // 128
                tail = rows - 128 * fw
                mab = mub[:]
                for oi, out_t in ((0, mu_out), (OUT_C, ls_out)):
                    if fw:
                        nc.sync.dma_start(
                            out=out_t.ap()[128 * b0:128 * b0 + 128 * fw, :]
                                .rearrange("(w p) f -> p w f", p=128),
                            in_=bass.AP(mab.tensor, mab.offset + oi,
                                        [mab.ap[0], [2 * OUT_C, fw],
                                         [1, OUT_C]]))
                    if tail:
                        nc.sync.dma_start(
                            out=out_t.ap()[128 * b0 + 128 * fw:
                                           128 * b0 + rows, :],
                            in_=mub[:tail, 2 * OUT_C * fw + oi:
                                    2 * OUT_C * fw + oi + OUT_C])

            for _ in range(reps):
                h_full = first_half()
                aggregation(h_full, epi2, bank_start=bank2_start,
                            bank_flush=flush2)

    nc.compile()
    return nc


TUNED_CFG = {"gbufs": 10, "sbufs": 10}


def build_in_maps(x, W1, b1, W_mu, b_mu, W_ls, b_ls, dinv, per_core):
    x = np.asarray(x)
    iota = np.tile(np.arange(128, dtype=np.float32), (128, 1)).astype(BF)
    ident = np.eye(128, dtype=np.float32).astype(BF)
    w1 = np.asarray(W1, np.float32).astype(BF)
    wmu = np.asarray(W_mu, np.float32).astype(BF)
    wls = np.asarray(W_ls, np.float32).astype(BF)
    b1t = np.tile(np.asarray(b1, np.float32), (128, 1))
    bmuls = np.tile(np.concatenate([np.asarray(b_mu, np.float32),
                                    np.asarray(b_ls, np.float32)]), (128, 1))
    XCOLS = NBLK * 128

    # replicated full x in table (chunk-major, assigned-position) order
    CB = CHUNK // 128 + 1
    XF = NU * CB * 128
    pa = np.stack([per_core[k][2] for k in range(N_CORES)])   # [cores, R]
    t = np.arange(N_NODES)
    jj, rem = t // CHUNK, t % CHUNK
    ks, ii = rem // UR, rem % UR
    node = ks * R + pa[ks, jj * UR + ii]
    xfull = x[node].astype(np.float32)
    dvfull = np.asarray(dinv, np.float32)[node]
    xTf = np.zeros((128, XF), dtype=BF)
    dinvf = np.ones((128, XF // 128), dtype=np.float32)
    for j in range(NU):
        seg = xfull[j * CHUNK:(j + 1) * CHUNK]
        xTf[:, CB * 128 * j:CB * 128 * j + CHUNK] = \
            np.ascontiguousarray(seg.T).astype(BF)
        pad = np.ones(CB * 128, np.float32)
        pad[:CHUNK] = dvfull[j * CHUNK:(j + 1) * CHUNK]
        dinvf[:, CB * j:CB * (j + 1)] = pad.reshape(CB, 128).T

    in_maps = []
    for k in range(N_CORES):
        idx16, dst128, perm = per_core[k]
        xk = x[R * k:R * (k + 1)][perm].astype(np.float32)
        xTk = np.zeros((128, XCOLS), dtype=BF)
        xTk[:, :R] = np.ascontiguousarray(xk.T).astype(BF)
        dv = dinv[R * k:R * (k + 1)][perm]
        padded = np.ones(NBLK * 128, dtype=np.float32)
        padded[:R] = dv
        dinv_blk = np.ascontiguousarray(padded.reshape(NBLK, 128).T)
        in_maps.append({
            "xT": xTk, "xTf": xTf, "dinvf_in": dinvf,
            "idx_in": idx16, "dst_in": dst128, "dinv_in": dinv_blk,
            "iota_in": iota, "ident_in": ident, "w1_in": w1, "wmu_in": wmu,
            "wls_in": wls, "b1_in": b1t, "bmuls_in": bmuls,
        })
    return in_maps


def kernel(x, edge_index, W1, b1, W_mu, b_mu, W_ls, b_ls):
    dinv, plan, per_core = _prep(np.asarray(edge_index))
    nc = _build(plan, cfg=TUNED_CFG)
    in_maps = build_in_maps(x, W1, b1, W_mu, b_mu, W_ls, b_ls, dinv, per_core)
    res = bass_utils.run_bass_kernel_spmd(nc, in_maps, core_ids=list(range(N_CORES)))
    mu = np.empty((N_NODES, OUT_C), dtype=np.float32)
    ls = np.empty((N_NODES, OUT_C), dtype=np.float32)
    for k in range(N_CORES):
        perm = per_core[k][2]
        mu[R * k + perm] = res.results[k]["mu_out"]
        ls[R * k + perm] = res.results[k]["ls_out"]
    return (mu, ls)

